# revision 9
# baseline (speedup 1.0000x reference)
"""MoE-routing (MANN-style) network on 8 Trainium2 NeuronCores.

Strategy: pure data parallel. Each core receives the FULL batch
(transposed, batch-rotated so that "its" 128-sample shard sits at
columns 0:128) and computes:
  - encoder (Linear+BN+ReLU x2) over the full batch (BatchNorm in
    training mode needs full-batch statistics; the rotation makes the
    stats permutation-invariant and keeps the program identical across
    cores -- SPMD with per-core data only),
  - gating MLP + softmax for its shard only,
  - the expensive expert-blended 3-layer generator for its shard only.

All weights are pre-rearranged on the host into the exact SBUF layouts
the TensorEngine wants, so every DMA is a fully contiguous stream.
Expert blending sum_e bc[b,e] * (x @ W_e) is computed with per-expert
matmuls (the blend weight is a per-partition scalar in batch-on-
partitions layout, so it fuses into one scalar_tensor_tensor per
expert).  Matmuls run as float32r (full-rate fp32 path on TRN2).
"""

import numpy as np

import concourse.bacc as bacc
import concourse.tile as tile
from concourse import masks, mybir
from concourse.bass_utils import run_bass_kernel_spmd

F32 = mybir.dt.float32
F32R = mybir.dt.float32r
ALU = mybir.AluOpType
ACTF = mybir.ActivationFunctionType
AX = mybir.AxisListType

B, DIN, L1W, L2W, E, GH, H, DOUT = 1024, 103, 64, 32, 8, 64, 256, 51
NCORES = 8
SH = B // NCORES  # 128 samples per core
EPS = 1e-5

import os

# matmul operand mode:
#   "f32"  - plain fp32 (exact, 4 cycles/col on PE)
#   "f32r" - tfloat32 (full-rate 1 cycle/col at N>=256, ~10-bit mantissa)
#   "bf16" - bfloat16 operands (full rate, halves weight DMA traffic)
MM_MODE = os.environ.get("KERNEL_MM_MODE", "f32")
BF16 = mybir.dt.bfloat16
MM_DT = {"f32": F32, "f32r": F32R, "bf16": BF16}[MM_MODE]


def _mm(nc, out, lhsT, rhs, start=True, stop=True):
    nc.tensor.matmul(out, lhsT, rhs, start=start, stop=stop)


def _elu(nc, pool, out, z, bias_ap=None, tag=""):
    """out = elu(z + bias). elu(x) = relu(x) + exp(min(x,0)) - 1."""
    p, f = z.shape[0], z.free_size()
    mt = pool.tile([p, f], F32, tag=f"elu_m{tag}")
    rt = pool.tile([p, f], F32, tag=f"elu_r{tag}")
    et = pool.tile([p, f], F32, tag=f"elu_e{tag}")
    if bias_ap is not None:
        nc.vector.tensor_scalar(mt, z, bias_ap, 0.0, ALU.add, ALU.min)
        nc.vector.tensor_scalar(rt, z, bias_ap, 0.0, ALU.add, ALU.max)
    else:
        nc.vector.tensor_scalar(mt, z, 0.0, None, ALU.min)
        nc.vector.tensor_scalar(rt, z, 0.0, None, ALU.max)
    nc.scalar.activation(et, mt, ACTF.Exp)
    # (exp(min) + (-1)) + relu
    nc.vector.scalar_tensor_tensor(out, et, -1.0, rt, ALU.add, ALU.add)


def _bn_apply_params(nc, pool, psum, width, nchunks, gamma_ap, beta_ap, eps_ap, tag):
    """Full-batch BN stats over psum [width, nchunks*512]; returns
    (scale, shift) APs [width, 1] s.t. bn(x) = scale*x + shift."""
    st = pool.tile([width, 6 * nchunks], F32, tag=f"bnst{tag}")
    for i in range(nchunks):
        nc.vector.bn_stats(st[:, 6 * i : 6 * (i + 1)], psum[:, 512 * i : 512 * (i + 1)])
    mv = pool.tile([width, 2], F32, tag=f"bnmv{tag}")
    nc.vector.bn_aggr(mv, st)
    sd = pool.tile([width, 1], F32, tag=f"bnsd{tag}")
    nc.scalar.activation(sd, mv[:, 1:2], ACTF.Sqrt, bias=eps_ap)  # sqrt(var+eps)
    rstd = pool.tile([width, 1], F32, tag=f"bnrs{tag}")
    nc.vector.reciprocal(rstd, sd)
    s = pool.tile([width, 1], F32, tag=f"bns{tag}")
    nc.vector.tensor_tensor(s, gamma_ap, rstd, ALU.mult)
    sm = pool.tile([width, 1], F32, tag=f"bnsm{tag}")
    nc.vector.tensor_tensor(sm, mv[:, 0:1], s, ALU.mult)
    t = pool.tile([width, 1], F32, tag=f"bnt{tag}")
    nc.vector.tensor_tensor(t, beta_ap, sm, ALU.subtract)  # beta - s*m
    return s, t


def _build_kernel(tc, d, out_ap):
    nc = tc.nc
    with (
        tc.tile_pool(name="const", bufs=1) as cp,
        tc.tile_pool(name="work", bufs=1) as wp,
        tc.tile_pool(name="psum", bufs=1, space="PSUM") as pp,
    ):
        # ---- constant loads (contiguous DMAs) ----
        ident = cp.tile([128, 128], F32)
        masks.make_identity(nc, ident)

        xT = cp.tile([DIN, B], MM_DT)
        nc.sync.dma_start(xT[:, 0:512], d["xT"][:, 0:512])
        nc.sync.dma_start(xT[:, 512:B], d["xT"][:, 512:B])
        w1T = cp.tile([DIN, L1W], MM_DT)
        nc.sync.dma_start(w1T, d["w1T"])
        pmat = cp.tile([64, 168], MM_DT)
        nc.sync.dma_start(pmat, d["pmat"])
        pvec = cp.tile([128, 9], F32)
        nc.sync.dma_start(pvec, d["pvec"])
        ebp = cp.tile([E, 2 * H + DOUT], MM_DT)
        nc.sync.dma_start(ebp, d["ebp"])
        ew1 = cp.tile([DIN, E * H], MM_DT)
        nc.sync.dma_start(ew1[:, 0:1024], d["ew1r"][:, 0:1024])
        nc.sync.dma_start(ew1[:, 1024:2048], d["ew1r"][:, 1024:2048])
        ew2 = cp.tile([128, 2 * E * H], MM_DT)
        for q in range(4):
            nc.sync.dma_start(
                ew2[:, 1024 * q : 1024 * (q + 1)], d["ew2r"][:, 1024 * q : 1024 * (q + 1)]
            )
        ew3 = cp.tile([128, 2 * E * DOUT], MM_DT)
        nc.sync.dma_start(ew3, d["ew3r"])

        # param views
        w2T = pmat[:, 0:32]          # [64, 32]
        gw1T = pmat[0:32, 32:96]     # [32, 64]
        gw2T = pmat[:, 96:160]       # [64, 64]
        gw3T = pmat[:, 160:168]      # [64, 8]
        gamma1, beta1 = pvec[0:L1W, 0:1], pvec[0:L1W, 1:2]
        gamma2, beta2 = pvec[0:L2W, 2:3], pvec[0:L2W, 3:4]
        gb1, gb2, gb3 = pvec[0:GH, 4:5], pvec[0:GH, 5:6], pvec[0:E, 6:7]
        eb1, eb2, eb3 = ebp[:, 0:H], ebp[:, H : 2 * H], ebp[:, 2 * H : 2 * H + DOUT]

        # scale last 3 input features by 100 (reference: x[:,100:103] *= 100)
        # partition offsets must be multiples of 32 -> scale rows 96:103 by a
        # per-partition vector (1 except rows 100..102 = 100)
        nc.scalar.mul(xT[96:DIN, :], xT[96:DIN, :], pvec[96:DIN, 8:9])

        # ---- encoder, full batch, transposed layout [feat, batch] ----
        e1p = pp.tile([L1W, B], F32, tag="acc")  # 2 banks of the 4-bank slot
        for j in range(2):
            _mm(nc, e1p[:, 512 * j : 512 * (j + 1)], w1T, xT[:, 512 * j : 512 * (j + 1)])
        s1, t1 = _bn_apply_params(nc, wp, e1p, L1W, 2, gamma1, beta1, pvec[0:L1W, 7:8], "1")
        e1 = wp.tile([L1W, B], MM_DT, name='dbg_e1', uniquify=False)
        nc.scalar.activation(e1, e1p, ACTF.Relu, bias=t1, scale=s1)

        e2p = pp.tile([L2W, B], F32, tag="acc")
        for j in range(2):
            _mm(nc, e2p[:, 512 * j : 512 * (j + 1)], w2T, e1[:, 512 * j : 512 * (j + 1)])
        s2, t2 = _bn_apply_params(nc, wp, e2p, L2W, 2, gamma2, beta2, pvec[0:L2W, 7:8], "2")
        # only this core's shard continues past BN2
        lat = wp.tile([L2W, SH], MM_DT, name='dbg_lat', uniquify=False)
        nc.scalar.activation(lat, e2p[:, 0:SH], ACTF.Relu, bias=t2, scale=s2)

        # ---- gating MLP (shard only), transposed layout ----
        g1p = pp.tile([GH, SH], F32, tag="tr")
        _mm(nc, g1p, gw1T, lat)
        g1 = wp.tile([GH, SH], MM_DT)
        _elu(nc, wp, g1, g1p, bias_ap=gb1, tag="g")
        g2p = pp.tile([GH, SH], F32, tag="tr")
        _mm(nc, g2p, gw2T, g1)
        g2 = wp.tile([GH, SH], MM_DT, name='dbg_g2', uniquify=False)
        _elu(nc, wp, g2, g2p, bias_ap=gb2, tag="g")
        g3p = pp.tile([E, SH], F32, tag="tr")
        _mm(nc, g3p, gw3T, g2)
        g3 = wp.tile([E, SH], F32, name='dbg_g3', uniquify=False)
        nc.scalar.activation(g3, g3p, ACTF.Identity, bias=gb3)

        # softmax over experts: transpose to [batch, E]
        g3tp = pp.tile([SH, E], F32, tag="tr")
        nc.tensor.transpose(g3tp, g3, ident[0:E, 0:E])
        negmx = wp.tile([SH, 1], F32)
        nc.vector.tensor_reduce(negmx, g3tp, AX.X, ALU.max, negate=True)
        ex = wp.tile([SH, E], F32)
        ssum = wp.tile([SH, 1], F32)
        nc.scalar.activation(ex, g3tp, ACTF.Exp, bias=negmx, accum_out=ssum)
        rs = wp.tile([SH, 1], F32)
        nc.vector.reciprocal(rs, ssum)
        bc = wp.tile([SH, E], F32, name='dbg_bc', uniquify=False)
        nc.vector.tensor_scalar(bc, ex, rs, None, ALU.mult)
        bctp = pp.tile([E, SH], F32, tag="tr")
        nc.tensor.transpose(bctp, bc, ident)
        bcT = wp.tile([E, SH], MM_DT)
        nc.scalar.copy(bcT, bctp)

        # ---- generator layer 1: y_e = xs @ ew1[e]  (batch on partitions) ----
        xsT = xT[:, 0:SH]  # [DIN, 128] lhsT for this shard
        y1 = pp.tile([SH, E * H], F32, tag="acc")  # 4 banks
        for j in range(4):
            _mm(nc, y1[:, 512 * j : 512 * (j + 1)], xsT, ew1[:, 512 * j : 512 * (j + 1)])
        b1p = pp.tile([SH, H], F32, tag="bias")
        _mm(nc, b1p, bcT, eb1)
        b1s = wp.tile([SH, H], F32, tag="bias_sb")
        nc.scalar.copy(b1s, b1p)
        acc1 = wp.tile([SH, H], F32, name='dbg_acc1', uniquify=False)
        nc.vector.scalar_tensor_tensor(acc1, y1[:, 0:H], bc[:, 0:1], b1s, ALU.mult, ALU.add)
        for e in range(1, E):
            nc.vector.scalar_tensor_tensor(
                acc1, y1[:, H * e : H * (e + 1)], bc[:, e : e + 1], acc1, ALU.mult, ALU.add
            )
        h1 = wp.tile([SH, H], F32, name='dbg_h1', uniquify=False)
        _elu(nc, wp, h1, acc1, tag="h")

        # transpose h1 -> lhsT chunks
        h1T = []
        for c in range(2):
            trp = pp.tile([128, 128], F32, tag="tr")
            nc.tensor.transpose(trp, h1[:, 128 * c : 128 * (c + 1)], ident)
            t_ = wp.tile([128, 128], MM_DT, tag=f"h1T{c}")
            nc.scalar.copy(t_, trp)
            h1T.append(t_)

        # ---- generator layer 2 ----
        y2 = pp.tile([SH, E * H], F32, tag="acc")
        for c in range(2):
            for j in range(4):
                _mm(
                    nc,
                    y2[:, 512 * j : 512 * (j + 1)],
                    h1T[c],
                    ew2[:, 2048 * c + 512 * j : 2048 * c + 512 * (j + 1)],
                    start=(c == 0),
                    stop=(c == 1),
                )
        b2p = pp.tile([SH, H], F32, tag="bias")
        _mm(nc, b2p, bcT, eb2)
        b2s = wp.tile([SH, H], F32, tag="bias_sb")
        nc.scalar.copy(b2s, b2p)
        acc2 = wp.tile([SH, H], F32, name='dbg_acc2', uniquify=False)
        nc.vector.scalar_tensor_tensor(acc2, y2[:, 0:H], bc[:, 0:1], b2s, ALU.mult, ALU.add)
        for e in range(1, E):
            nc.vector.scalar_tensor_tensor(
                acc2, y2[:, H * e : H * (e + 1)], bc[:, e : e + 1], acc2, ALU.mult, ALU.add
            )
        h2 = wp.tile([SH, H], F32, name='dbg_h2', uniquify=False)
        _elu(nc, wp, h2, acc2, tag="h")

        h2T = []
        for c in range(2):
            trp = pp.tile([128, 128], F32, tag="tr")
            nc.tensor.transpose(trp, h2[:, 128 * c : 128 * (c + 1)], ident)
            t_ = wp.tile([128, 128], MM_DT, tag=f"h2T{c}")
            nc.scalar.copy(t_, trp)
            h2T.append(t_)

        # ---- generator layer 3 (all experts side by side, N = 8*51) ----
        oP = pp.tile([SH, E * DOUT], F32, tag="tr")
        for c in range(2):
            _mm(
                nc,
                oP,
                h2T[c],
                ew3[:, E * DOUT * c : E * DOUT * (c + 1)],
                start=(c == 0),
                stop=(c == 1),
            )
        b3p = pp.tile([SH, DOUT], F32, tag="bias")
        _mm(nc, b3p, bcT, eb3)
        b3s = wp.tile([SH, DOUT], F32, tag="bias_sb3")
        nc.scalar.copy(b3s, b3p)
        osb = wp.tile([SH, DOUT], F32)
        nc.vector.scalar_tensor_tensor(
            osb, oP[:, 0:DOUT], bc[:, 0:1], b3s, ALU.mult, ALU.add
        )
        for e in range(1, E):
            nc.vector.scalar_tensor_tensor(
                osb, oP[:, DOUT * e : DOUT * (e + 1)], bc[:, e : e + 1], osb, ALU.mult, ALU.add
            )
        nc.sync.dma_start(out_ap, osb)


def build_program():
    nc = bacc.Bacc("TRN2", target_bir_lowering=False, debug=False, num_devices=NCORES)
    d = {}
    for name, shape, dt_ in [
        ("xT", (DIN, B), MM_DT),
        ("w1T", (DIN, L1W), MM_DT),
        ("pmat", (64, 168), MM_DT),
        ("pvec", (128, 9), F32),
        ("ew1r", (DIN, E * H), MM_DT),
        ("ew2r", (128, 2 * E * H), MM_DT),
        ("ew3r", (128, 2 * E * DOUT), MM_DT),
        ("ebp", (E, 2 * H + DOUT), MM_DT),
    ]:
        d[name] = nc.dram_tensor(name, list(shape), dt_, kind="ExternalInput").ap()
    out_ap = nc.dram_tensor("out", [SH, DOUT], F32, kind="ExternalOutput").ap()
    with tile.TileContext(nc) as tc:
        _build_kernel(tc, d, out_ap)
    nc.compile()
    return nc


def prep_in_maps(inputs):
    f = np.float32
    if MM_MODE == "bf16":
        import ml_dtypes

        mmf = ml_dtypes.bfloat16
    else:
        mmf = f

    def c(a):
        return np.ascontiguousarray(a, dtype=mmf)

    x = np.asarray(inputs["x"], dtype=f)
    pmat_dt = mmf
    w1T = c(np.asarray(inputs["w1"]).T)
    pmat = np.zeros((64, 168), pmat_dt)
    pmat[:, 0:32] = np.asarray(inputs["w2"]).T          # [64, 32]
    pmat[0:32, 32:96] = np.asarray(inputs["gw1"]).T     # [32, 64]
    pmat[:, 96:160] = np.asarray(inputs["gw2"]).T       # [64, 64]
    pmat[:, 160:168] = np.asarray(inputs["gw3"]).T      # [64, 8]
    pvec = np.zeros((128, 9), f)
    pvec[:, 7] = EPS
    pvec[:, 8] = 1.0
    pvec[100:103, 8] = 100.0
    pvec[0:64, 0] = inputs["gamma1"]
    pvec[0:64, 1] = inputs["beta1"]
    pvec[0:32, 2] = inputs["gamma2"]
    pvec[0:32, 3] = inputs["beta2"]
    pvec[0:64, 4] = inputs["gb1"]
    pvec[0:64, 5] = inputs["gb2"]
    pvec[0:8, 6] = inputs["gb3"]
    ew1r = c(np.asarray(inputs["ew1"]).transpose(1, 0, 2).reshape(DIN, E * H))
    ew2r = c(
        np.asarray(inputs["ew2"])
        .reshape(E, 2, 128, H)
        .transpose(2, 1, 0, 3)
        .reshape(128, 2 * E * H)
    )
    ew3r = c(
        np.asarray(inputs["ew3"])
        .reshape(E, 2, 128, DOUT)
        .transpose(2, 1, 0, 3)
        .reshape(128, 2 * E * DOUT)
    )
    ebp = c(np.concatenate([inputs["eb1"], inputs["eb2"], inputs["eb3"]], axis=1))

    shared = {
        "w1T": w1T, "pmat": pmat, "pvec": pvec,
        "ew1r": ew1r, "ew2r": ew2r, "ew3r": ew3r, "ebp": ebp,
    }
    in_maps = []
    for core in range(NCORES):
        xr = np.roll(x, -core * SH, axis=0)  # this core's shard -> rows 0:SH
        in_maps.append({**shared, "xT": c(xr.T)})
    return in_maps


_prog = None


def _get_program():
    global _prog
    if _prog is None:
        _prog = build_program()
    return _prog


def kernel(**inputs) -> np.ndarray:
    nc = _get_program()
    in_maps = prep_in_maps(inputs)
    res = run_bass_kernel_spmd(nc, in_maps, core_ids=list(range(NCORES)))
    return np.concatenate(
        [np.asarray(res.results[cid]["out"]) for cid in range(NCORES)], axis=0
    )


# revision 12
# speedup vs baseline: 1.1339x; 1.1339x over previous
"""MoE-routing (MANN-style) network on 8 Trainium2 NeuronCores.

Strategy: pure data parallel. Each core receives the FULL batch
(transposed, batch-rotated so that "its" 128-sample shard sits at
columns 0:128) and computes:
  - encoder (Linear+BN+ReLU x2) over the full batch (BatchNorm in
    training mode needs full-batch statistics; the rotation makes the
    stats permutation-invariant and keeps the program identical across
    cores -- SPMD with per-core data only),
  - gating MLP + softmax for its shard only,
  - the expensive expert-blended 3-layer generator for its shard only.

All weights are pre-rearranged on the host into the exact SBUF layouts
the TensorEngine wants, so every DMA is a fully contiguous stream.
Expert blending sum_e bc[b,e] * (x @ W_e) is computed with per-expert
matmuls (the blend weight is a per-partition scalar in batch-on-
partitions layout, so it fuses into one scalar_tensor_tensor per
expert).  Matmuls run as float32r (full-rate fp32 path on TRN2).
"""

import numpy as np

import concourse.bacc as bacc
import concourse.tile as tile
from concourse import masks, mybir
from concourse.bass_utils import run_bass_kernel_spmd

F32 = mybir.dt.float32
F32R = mybir.dt.float32r
ALU = mybir.AluOpType
ACTF = mybir.ActivationFunctionType
AX = mybir.AxisListType

B, DIN, L1W, L2W, E, GH, H, DOUT = 1024, 103, 64, 32, 8, 64, 256, 51
NCORES = 8
SH = B // NCORES  # 128 samples per core
EPS = 1e-5

import os

# matmul operand mode:
#   "f32"  - plain fp32 (exact, 4 cycles/col on PE)
#   "f32r" - tfloat32 (full-rate 1 cycle/col at N>=256, ~10-bit mantissa)
#   "bf16" - bfloat16 operands (full rate, halves weight DMA traffic)
MM_MODE = os.environ.get("KERNEL_MM_MODE", "f32")
BF16 = mybir.dt.bfloat16
MM_DT = {"f32": F32, "f32r": F32R, "bf16": BF16}[MM_MODE]


def _mm(nc, out, lhsT, rhs, start=True, stop=True):
    nc.tensor.matmul(out, lhsT, rhs, start=start, stop=stop)


def _elu(nc, pool, out, z, bias_ap=None, tag=""):
    """out = elu(z + bias). elu(x) = relu(x) + exp(min(x,0)) - 1."""
    p, f = z.shape[0], z.free_size()
    mt = pool.tile([p, f], F32, tag=f"elu_m{tag}")
    rt = pool.tile([p, f], F32, tag=f"elu_r{tag}")
    et = pool.tile([p, f], F32, tag=f"elu_e{tag}")
    if bias_ap is not None:
        nc.vector.tensor_scalar(mt, z, bias_ap, 0.0, ALU.add, ALU.min)
        nc.vector.tensor_scalar(rt, z, bias_ap, 0.0, ALU.add, ALU.max)
    else:
        nc.vector.tensor_scalar(mt, z, 0.0, None, ALU.min)
        nc.vector.tensor_scalar(rt, z, 0.0, None, ALU.max)
    nc.scalar.activation(et, mt, ACTF.Exp)
    # (exp(min) + (-1)) + relu
    nc.vector.scalar_tensor_tensor(out, et, -1.0, rt, ALU.add, ALU.add)


def _bn_apply_params(nc, pool, psum, width, nchunks, gamma_ap, beta_ap, eps_ap, tag):
    """Full-batch BN stats over psum [width, nchunks*512]; returns
    (scale, shift) APs [width, 1] s.t. bn(x) = scale*x + shift."""
    st = pool.tile([width, 6 * nchunks], F32, tag=f"bnst{tag}")
    for i in range(nchunks):
        nc.vector.bn_stats(st[:, 6 * i : 6 * (i + 1)], psum[:, 512 * i : 512 * (i + 1)])
    mv = pool.tile([width, 2], F32, tag=f"bnmv{tag}")
    nc.vector.bn_aggr(mv, st)
    sd = pool.tile([width, 1], F32, tag=f"bnsd{tag}")
    nc.scalar.activation(sd, mv[:, 1:2], ACTF.Sqrt, bias=eps_ap)  # sqrt(var+eps)
    rstd = pool.tile([width, 1], F32, tag=f"bnrs{tag}")
    nc.vector.reciprocal(rstd, sd)
    s = pool.tile([width, 1], F32, tag=f"bns{tag}")
    nc.vector.tensor_tensor(s, gamma_ap, rstd, ALU.mult)
    sm = pool.tile([width, 1], F32, tag=f"bnsm{tag}")
    nc.vector.tensor_tensor(sm, mv[:, 0:1], s, ALU.mult)
    t = pool.tile([width, 1], F32, tag=f"bnt{tag}")
    nc.vector.tensor_tensor(t, beta_ap, sm, ALU.subtract)  # beta - s*m
    return s, t


def _build_kernel(tc, d, out_ap):
    nc = tc.nc
    with (
        tc.tile_pool(name="const", bufs=1) as cp,
        tc.tile_pool(name="work", bufs=1) as wp,
        tc.tile_pool(name="psum", bufs=1, space="PSUM") as pp,
    ):
        # ---- constant loads (contiguous DMAs) ----
        ident = cp.tile([128, 128], F32)
        masks.make_identity(nc, ident)

        # All big tensors are padded to 128 partitions and loaded with ONE
        # dense dma_start each: HBM->SBUF transfers only fan out across the
        # 16 SDMA engines when the destination spans all 128 partitions --
        # sub-128-partition transfers drain through a single engine at
        # ~25 GB/s instead of ~340 GB/s.  Two HWDGE queues (sync + scalar)
        # drain in parallel.
        xTf = cp.tile([128, B], MM_DT)
        nc.sync.dma_start(xTf, d["xT"])
        xT = xTf[0:DIN, :]
        w1Tf = cp.tile([128, L1W], MM_DT)
        nc.scalar.dma_start(w1Tf, d["w1T"])
        w1T = w1Tf[0:DIN, :]
        pmatf = cp.tile([128, 168], MM_DT)
        nc.scalar.dma_start(pmatf, d["pmat"])
        pmat = pmatf[0:64, :]
        pvec = cp.tile([128, 9], F32)
        nc.scalar.dma_start(pvec, d["pvec"])
        ebp = cp.tile([E, 2 * H + DOUT], MM_DT)
        nc.scalar.dma_start(ebp, d["ebp"])
        ew1f = cp.tile([128, E * H], MM_DT)
        nc.scalar.dma_start(ew1f, d["ew1r"])
        ew1 = ew1f[0:DIN, :]
        ew2 = cp.tile([128, 2 * E * H], MM_DT)
        for q in range(4):
            nc.sync.dma_start(
                ew2[:, 1024 * q : 1024 * (q + 1)], d["ew2r"][:, 1024 * q : 1024 * (q + 1)]
            )
        ew3 = cp.tile([128, 2 * E * DOUT], MM_DT)
        nc.scalar.dma_start(ew3, d["ew3r"])

        # param views
        w2T = pmat[:, 0:32]          # [64, 32]
        gw1T = pmat[0:32, 32:96]     # [32, 64]
        gw2T = pmat[:, 96:160]       # [64, 64]
        gw3T = pmat[:, 160:168]      # [64, 8]
        gamma1, beta1 = pvec[0:L1W, 0:1], pvec[0:L1W, 1:2]
        gamma2, beta2 = pvec[0:L2W, 2:3], pvec[0:L2W, 3:4]
        gb1, gb2, gb3 = pvec[0:GH, 4:5], pvec[0:GH, 5:6], pvec[0:E, 6:7]
        eb1, eb2, eb3 = ebp[:, 0:H], ebp[:, H : 2 * H], ebp[:, 2 * H : 2 * H + DOUT]

        # scale last 3 input features by 100 (reference: x[:,100:103] *= 100)
        # partition offsets must be multiples of 32 -> scale rows 96:103 by a
        # per-partition vector (1 except rows 100..102 = 100)
        nc.scalar.mul(xT[96:DIN, :], xT[96:DIN, :], pvec[96:DIN, 8:9])

        # ---- encoder, full batch, transposed layout [feat, batch] ----
        e1p = pp.tile([L1W, B], F32, tag="acc")  # 2 banks of the 4-bank slot
        for j in range(2):
            _mm(nc, e1p[:, 512 * j : 512 * (j + 1)], w1T, xT[:, 512 * j : 512 * (j + 1)])
        s1, t1 = _bn_apply_params(nc, wp, e1p, L1W, 2, gamma1, beta1, pvec[0:L1W, 7:8], "1")
        e1 = wp.tile([L1W, B], MM_DT, name='dbg_e1', uniquify=False)
        nc.scalar.activation(e1, e1p, ACTF.Relu, bias=t1, scale=s1)

        e2p = pp.tile([L2W, B], F32, tag="acc")
        for j in range(2):
            _mm(nc, e2p[:, 512 * j : 512 * (j + 1)], w2T, e1[:, 512 * j : 512 * (j + 1)])
        s2, t2 = _bn_apply_params(nc, wp, e2p, L2W, 2, gamma2, beta2, pvec[0:L2W, 7:8], "2")
        # only this core's shard continues past BN2
        lat = wp.tile([L2W, SH], MM_DT, name='dbg_lat', uniquify=False)
        nc.scalar.activation(lat, e2p[:, 0:SH], ACTF.Relu, bias=t2, scale=s2)

        # ---- gating MLP (shard only), transposed layout ----
        g1p = pp.tile([GH, SH], F32, tag="tr")
        _mm(nc, g1p, gw1T, lat)
        g1 = wp.tile([GH, SH], MM_DT)
        _elu(nc, wp, g1, g1p, bias_ap=gb1, tag="g")
        g2p = pp.tile([GH, SH], F32, tag="tr")
        _mm(nc, g2p, gw2T, g1)
        g2 = wp.tile([GH, SH], MM_DT, name='dbg_g2', uniquify=False)
        _elu(nc, wp, g2, g2p, bias_ap=gb2, tag="g")
        g3p = pp.tile([E, SH], F32, tag="tr")
        _mm(nc, g3p, gw3T, g2)
        g3 = wp.tile([E, SH], F32, name='dbg_g3', uniquify=False)
        nc.scalar.activation(g3, g3p, ACTF.Identity, bias=gb3)

        # softmax over experts: transpose to [batch, E]
        g3tp = pp.tile([SH, E], F32, tag="tr")
        nc.tensor.transpose(g3tp, g3, ident[0:E, 0:E])
        negmx = wp.tile([SH, 1], F32)
        nc.vector.tensor_reduce(negmx, g3tp, AX.X, ALU.max, negate=True)
        ex = wp.tile([SH, E], F32)
        ssum = wp.tile([SH, 1], F32)
        nc.scalar.activation(ex, g3tp, ACTF.Exp, bias=negmx, accum_out=ssum)
        rs = wp.tile([SH, 1], F32)
        nc.vector.reciprocal(rs, ssum)
        bc = wp.tile([SH, E], F32, name='dbg_bc', uniquify=False)
        nc.vector.tensor_scalar(bc, ex, rs, None, ALU.mult)
        bctp = pp.tile([E, SH], F32, tag="tr")
        nc.tensor.transpose(bctp, bc, ident)
        bcT = wp.tile([E, SH], MM_DT)
        nc.scalar.copy(bcT, bctp)

        # ---- generator layer 1: y_e = xs @ ew1[e]  (batch on partitions) ----
        xsT = xT[:, 0:SH]  # [DIN, 128] lhsT for this shard
        b1p = pp.tile([SH, H], F32, tag="bias")
        _mm(nc, b1p, bcT, eb1)
        b1s = wp.tile([SH, H], F32, tag="bias_sb")
        nc.scalar.copy(b1s, b1p)
        y1 = pp.tile([SH, E * H], F32, tag="acc")  # 4 banks
        acc1 = wp.tile([SH, H], F32, name='dbg_acc1', uniquify=False)
        for j in range(4):
            _mm(nc, y1[:, 512 * j : 512 * (j + 1)], xsT, ew1[:, 512 * j : 512 * (j + 1)])
            for e in (2 * j, 2 * j + 1):
                prev = b1s if e == 0 else acc1
                nc.vector.scalar_tensor_tensor(
                    acc1, y1[:, H * e : H * (e + 1)], bc[:, e : e + 1], prev,
                    ALU.mult, ALU.add,
                )
        h1 = wp.tile([SH, H], F32, name='dbg_h1', uniquify=False)
        _elu(nc, wp, h1, acc1, tag="h")

        # transpose h1 -> lhsT chunks
        h1T = []
        for c in range(2):
            trp = pp.tile([128, 128], F32, tag="tr")
            nc.tensor.transpose(trp, h1[:, 128 * c : 128 * (c + 1)], ident)
            t_ = wp.tile([128, 128], MM_DT, tag=f"h1T{c}")
            nc.scalar.copy(t_, trp)
            h1T.append(t_)

        # ---- generator layer 2 ----
        b2p = pp.tile([SH, H], F32, tag="bias")
        _mm(nc, b2p, bcT, eb2)
        b2s = wp.tile([SH, H], F32, tag="bias_sb")
        nc.scalar.copy(b2s, b2p)
        y2 = pp.tile([SH, E * H], F32, tag="acc")
        acc2 = wp.tile([SH, H], F32, name='dbg_acc2', uniquify=False)
        # j-outer / c-inner: psum bank j completes after its two chained
        # matmuls, so the DVE blend for experts 2j, 2j+1 overlaps the
        # remaining matmuls instead of trailing them.
        for j in range(4):
            for c in range(2):
                _mm(
                    nc,
                    y2[:, 512 * j : 512 * (j + 1)],
                    h1T[c],
                    ew2[:, 1024 * j + 512 * c : 1024 * j + 512 * (c + 1)],
                    start=(c == 0),
                    stop=(c == 1),
                )
            for e in (2 * j, 2 * j + 1):
                prev = b2s if e == 0 else acc2
                nc.vector.scalar_tensor_tensor(
                    acc2, y2[:, H * e : H * (e + 1)], bc[:, e : e + 1], prev,
                    ALU.mult, ALU.add,
                )
        h2 = wp.tile([SH, H], F32, name='dbg_h2', uniquify=False)
        _elu(nc, wp, h2, acc2, tag="h")

        h2T = []
        for c in range(2):
            trp = pp.tile([128, 128], F32, tag="tr")
            nc.tensor.transpose(trp, h2[:, 128 * c : 128 * (c + 1)], ident)
            t_ = wp.tile([128, 128], MM_DT, tag=f"h2T{c}")
            nc.scalar.copy(t_, trp)
            h2T.append(t_)

        # ---- generator layer 3 (all experts side by side, N = 8*51) ----
        oP = pp.tile([SH, E * DOUT], F32, tag="tr")
        for c in range(2):
            _mm(
                nc,
                oP,
                h2T[c],
                ew3[:, E * DOUT * c : E * DOUT * (c + 1)],
                start=(c == 0),
                stop=(c == 1),
            )
        b3p = pp.tile([SH, DOUT], F32, tag="bias")
        _mm(nc, b3p, bcT, eb3)
        b3s = wp.tile([SH, DOUT], F32, tag="bias_sb3")
        nc.scalar.copy(b3s, b3p)
        osb = wp.tile([SH, DOUT], F32)
        nc.vector.scalar_tensor_tensor(
            osb, oP[:, 0:DOUT], bc[:, 0:1], b3s, ALU.mult, ALU.add
        )
        for e in range(1, E):
            nc.vector.scalar_tensor_tensor(
                osb, oP[:, DOUT * e : DOUT * (e + 1)], bc[:, e : e + 1], osb, ALU.mult, ALU.add
            )
        nc.sync.dma_start(out_ap, osb)


def build_program():
    nc = bacc.Bacc("TRN2", target_bir_lowering=False, debug=False, num_devices=NCORES)
    d = {}
    for name, shape, dt_ in [
        ("xT", (128, B), MM_DT),
        ("w1T", (128, L1W), MM_DT),
        ("pmat", (128, 168), MM_DT),
        ("pvec", (128, 9), F32),
        ("ew1r", (128, E * H), MM_DT),
        ("ew2r", (128, 2 * E * H), MM_DT),
        ("ew3r", (128, 2 * E * DOUT), MM_DT),
        ("ebp", (E, 2 * H + DOUT), MM_DT),
    ]:
        d[name] = nc.dram_tensor(name, list(shape), dt_, kind="ExternalInput").ap()
    out_ap = nc.dram_tensor("out", [SH, DOUT], F32, kind="ExternalOutput").ap()
    with tile.TileContext(nc) as tc:
        _build_kernel(tc, d, out_ap)
    nc.compile()
    return nc


def prep_in_maps(inputs):
    f = np.float32
    if MM_MODE == "bf16":
        import ml_dtypes

        mmf = ml_dtypes.bfloat16
    else:
        mmf = f

    def c(a):
        return np.ascontiguousarray(a, dtype=mmf)

    x = np.asarray(inputs["x"], dtype=f)

    def pad128(a):
        out = np.zeros((128, a.shape[1]), a.dtype)
        out[: a.shape[0]] = a
        return out

    w1T = pad128(c(np.asarray(inputs["w1"]).T))
    pmat = np.zeros((128, 168), mmf)
    pmat[0:64, 0:32] = np.asarray(inputs["w2"]).T       # [64, 32]
    pmat[0:32, 32:96] = np.asarray(inputs["gw1"]).T     # [32, 64]
    pmat[0:64, 96:160] = np.asarray(inputs["gw2"]).T    # [64, 64]
    pmat[0:64, 160:168] = np.asarray(inputs["gw3"]).T   # [64, 8]
    pvec = np.zeros((128, 9), f)
    pvec[:, 7] = EPS
    pvec[:, 8] = 1.0
    pvec[100:103, 8] = 100.0
    pvec[0:64, 0] = inputs["gamma1"]
    pvec[0:64, 1] = inputs["beta1"]
    pvec[0:32, 2] = inputs["gamma2"]
    pvec[0:32, 3] = inputs["beta2"]
    pvec[0:64, 4] = inputs["gb1"]
    pvec[0:64, 5] = inputs["gb2"]
    pvec[0:8, 6] = inputs["gb3"]
    ew1r = pad128(c(np.asarray(inputs["ew1"]).transpose(1, 0, 2).reshape(DIN, E * H)))
    # [e, h, k] -> [p, j, c, i2, k] with e = 2j+i2, h = 128c+p: matmul (j, c)
    # reads the contiguous column block j*1024 + c*512.
    ew2r = c(
        np.asarray(inputs["ew2"])
        .reshape(4, 2, 2, 128, H)
        .transpose(3, 0, 2, 1, 4)
        .reshape(128, 2 * E * H)
    )
    ew3r = c(
        np.asarray(inputs["ew3"])
        .reshape(E, 2, 128, DOUT)
        .transpose(2, 1, 0, 3)
        .reshape(128, 2 * E * DOUT)
    )
    ebp = c(np.concatenate([inputs["eb1"], inputs["eb2"], inputs["eb3"]], axis=1))

    shared = {
        "w1T": w1T, "pmat": pmat, "pvec": pvec,
        "ew1r": ew1r, "ew2r": ew2r, "ew3r": ew3r, "ebp": ebp,
    }
    in_maps = []
    for core in range(NCORES):
        xr = np.roll(x, -core * SH, axis=0)  # this core's shard -> rows 0:SH
        in_maps.append({**shared, "xT": pad128(c(xr.T))})
    return in_maps


_prog = None


def _get_program():
    global _prog
    if _prog is None:
        _prog = build_program()
    return _prog


def kernel(**inputs) -> np.ndarray:
    nc = _get_program()
    in_maps = prep_in_maps(inputs)
    res = run_bass_kernel_spmd(nc, in_maps, core_ids=list(range(NCORES)))
    return np.concatenate(
        [np.asarray(res.results[cid]["out"]) for cid in range(NCORES)], axis=0
    )


# revision 22
# speedup vs baseline: 1.3605x; 1.1998x over previous
"""MoE-routing (MANN-style) network on 8 Trainium2 NeuronCores.

Strategy: pure data parallel. Each core receives the FULL batch
(transposed, batch-rotated so that "its" 128-sample shard sits at
columns 0:128) and computes:
  - encoder (Linear+BN+ReLU x2) over the full batch (BatchNorm in
    training mode needs full-batch statistics; the rotation makes the
    stats permutation-invariant and keeps the program identical across
    cores -- SPMD with per-core data only),
  - gating MLP + softmax for its shard only,
  - the expensive expert-blended 3-layer generator for its shard only.

All weights are pre-rearranged on the host into the exact SBUF layouts
the TensorEngine wants, so every DMA is a fully contiguous stream.
Expert blending sum_e bc[b,e] * (x @ W_e) is computed with per-expert
matmuls (the blend weight is a per-partition scalar in batch-on-
partitions layout, so it fuses into one scalar_tensor_tensor per
expert).  Matmuls run as float32r (full-rate fp32 path on TRN2).
"""

import numpy as np

import concourse.bacc as bacc
import concourse.tile as tile
from concourse import masks, mybir
from concourse.bass_utils import run_bass_kernel_spmd

F32 = mybir.dt.float32
F32R = mybir.dt.float32r
ALU = mybir.AluOpType
ACTF = mybir.ActivationFunctionType
AX = mybir.AxisListType

B, DIN, L1W, L2W, E, GH, H, DOUT = 1024, 103, 64, 32, 8, 64, 256, 51
NCORES = 8
SH = B // NCORES  # 128 samples per core
EPS = 1e-5

import os

# matmul operand mode:
#   "f32"  - plain fp32 (exact, 4 cycles/col on PE)
#   "f32r" - tfloat32 (full-rate 1 cycle/col at N>=256, ~10-bit mantissa)
#   "bf16" - bfloat16 operands (full rate, halves weight DMA traffic)
MM_MODE = os.environ.get("KERNEL_MM_MODE", "f32r")
BF16 = mybir.dt.bfloat16
MM_DT = {"f32": F32, "f32r": F32R, "bf16": BF16}[MM_MODE]


def _mm(nc, out, lhsT, rhs, start=True, stop=True):
    nc.tensor.matmul(out, lhsT, rhs, start=start, stop=stop)


def _elu(nc, pool, out, z, bias_ap=None, tag="", on_gpsimd=False):
    """out = elu(z + bias). elu(x) = relu(x) + exp(min(x,0)) - 1."""
    p, f = z.shape[0], z.free_size()
    mt = pool.tile([p, f], F32, tag=f"elu_m{tag}")
    rt = pool.tile([p, f], F32, tag=f"elu_r{tag}")
    et = pool.tile([p, f], F32, tag=f"elu_e{tag}")
    eng = nc.gpsimd if on_gpsimd else nc.vector  # gpsimd only for SBUF inputs
    if bias_ap is not None:
        eng.tensor_scalar(mt, z, bias_ap, 0.0, ALU.add, ALU.min)
        eng.tensor_scalar(rt, z, bias_ap, 0.0, ALU.add, ALU.max)
    else:
        eng.tensor_scalar(mt, z, 0.0, None, ALU.min)
        eng.tensor_scalar(rt, z, 0.0, None, ALU.max)
    nc.scalar.activation(et, mt, ACTF.Exp)
    # (exp(min) + (-1)) + relu
    nc.vector.scalar_tensor_tensor(out, et, -1.0, rt, ALU.add, ALU.add)


def _bn_apply_params(nc, pool, psum, width, nchunks, gamma_ap, beta_ap, eps_ap, tag):
    """Full-batch BN stats over psum [width, nchunks*512]; returns
    (scale, shift) APs [width, 1] s.t. bn(x) = scale*x + shift."""
    st = pool.tile([width, 6 * nchunks], F32, tag=f"bnst{tag}")
    for i in range(nchunks):
        nc.vector.bn_stats(st[:, 6 * i : 6 * (i + 1)], psum[:, 512 * i : 512 * (i + 1)])
    mv = pool.tile([width, 2], F32, tag=f"bnmv{tag}")
    nc.vector.bn_aggr(mv, st)
    sd = pool.tile([width, 1], F32, tag=f"bnsd{tag}")
    nc.scalar.activation(sd, mv[:, 1:2], ACTF.Sqrt, bias=eps_ap)  # sqrt(var+eps)
    rstd = pool.tile([width, 1], F32, tag=f"bnrs{tag}")
    nc.vector.reciprocal(rstd, sd)
    s = pool.tile([width, 1], F32, tag=f"bns{tag}")
    nc.vector.tensor_tensor(s, gamma_ap, rstd, ALU.mult)
    sm = pool.tile([width, 1], F32, tag=f"bnsm{tag}")
    nc.vector.tensor_tensor(sm, mv[:, 0:1], s, ALU.mult)
    t = pool.tile([width, 1], F32, tag=f"bnt{tag}")
    nc.vector.tensor_tensor(t, beta_ap, sm, ALU.subtract)  # beta - s*m
    return s, t


def _build_kernel(tc, d, out_ap):
    nc = tc.nc
    with (
        tc.tile_pool(name="const", bufs=1) as cp,
        tc.tile_pool(name="work", bufs=1) as wp,
        tc.tile_pool(name="psum", bufs=1, space="PSUM") as pp,
    ):
        # ---- constant loads (contiguous DMAs) ----
        ident = cp.tile([128, 128], F32)
        masks.make_identity(nc, ident)

        # All big tensors are padded to 128 partitions and loaded with ONE
        # dense dma_start each: HBM->SBUF transfers only fan out across the
        # 16 SDMA engines when the destination spans all 128 partitions --
        # sub-128-partition transfers drain through a single engine at
        # ~25 GB/s instead of ~340 GB/s.  Two HWDGE queues (sync + scalar)
        # drain in parallel.
        xTf = cp.tile([128, B], MM_DT)
        nc.sync.dma_start(xTf[:, 0:512], d["xT"][:, 0:512])
        nc.sync.dma_start(xTf[:, 512:B], d["xT"][:, 512:B])
        xT = xTf[0:DIN, :]
        w1Tf = cp.tile([128, L1W], MM_DT)
        nc.scalar.dma_start(w1Tf, d["w1T"])
        w1T = w1Tf[0:DIN, :]
        pmatf = cp.tile([128, 168], MM_DT)
        nc.scalar.dma_start(pmatf, d["pmat"])
        pmat = pmatf[0:64, :]
        pvec = cp.tile([128, 9], F32)
        nc.scalar.dma_start(pvec, d["pvec"])
        eb2f = cp.tile([1, E * H], MM_DT)
        nc.scalar.dma_start(eb2f, d["eb2f"])
        eb3f = cp.tile([1, E * DOUT + SH], MM_DT)
        nc.scalar.dma_start(eb3f, d["eb3f"])
        ones_row = eb3f[:, E * DOUT : E * DOUT + SH]
        ew1f = cp.tile([128, E * H], MM_DT)
        nc.scalar.dma_start(ew1f, d["ew1r"])
        ew1 = ew1f[0:DIN, :]
        ew2 = cp.tile([128, 2 * E * H], MM_DT)
        for q in range(4):
            nc.sync.dma_start(
                ew2[:, 1024 * q : 1024 * (q + 1)], d["ew2r"][:, 1024 * q : 1024 * (q + 1)]
            )
        ew3 = cp.tile([128, 2 * E * DOUT], MM_DT)
        nc.scalar.dma_start(ew3, d["ew3r"])

        # param views
        w2T = pmat[:, 0:32]          # [64, 32]
        gw1T = pmat[0:32, 32:96]     # [32, 64]
        gw2T = pmat[:, 96:160]       # [64, 64]
        gw3T = pmat[:, 160:168]      # [64, 8]
        gamma1, beta1 = pvec[0:L1W, 0:1], pvec[0:L1W, 1:2]
        gamma2, beta2 = pvec[0:L2W, 2:3], pvec[0:L2W, 3:4]
        gb1, gb2, gb3 = pvec[0:GH, 4:5], pvec[0:GH, 5:6], pvec[0:E, 6:7]

        # scale last 3 input features by 100 (reference: x[:,100:103] *= 100)
        # partition offsets must be multiples of 32 -> scale rows 96:103 by a
        # per-partition vector (1 except rows 100..102 = 100); per column
        # chunk so encoder matmul j0 can start as soon as chunk 0 landed
        nc.scalar.mul(xT[96:DIN, 0:512], xT[96:DIN, 0:512], pvec[96:DIN, 8:9])
        nc.scalar.mul(xT[96:DIN, 512:B], xT[96:DIN, 512:B], pvec[96:DIN, 8:9])

        # ---- encoder, full batch, transposed layout [feat, batch] ----
        e1p = pp.tile([L1W, B], F32, tag="acc")  # 2 banks of the 4-bank slot
        for j in range(2):
            _mm(nc, e1p[:, 512 * j : 512 * (j + 1)], w1T, xT[:, 512 * j : 512 * (j + 1)])
        s1, t1 = _bn_apply_params(nc, wp, e1p, L1W, 2, gamma1, beta1, pvec[0:L1W, 7:8], "1")
        e1 = wp.tile([L1W, B], MM_DT, name='dbg_e1', uniquify=False)
        nc.scalar.activation(e1, e1p, ACTF.Relu, bias=t1, scale=s1)

        e2p = pp.tile([L2W, B], F32, tag="acc")
        for j in range(2):
            _mm(nc, e2p[:, 512 * j : 512 * (j + 1)], w2T, e1[:, 512 * j : 512 * (j + 1)])
        s2, t2 = _bn_apply_params(nc, wp, e2p, L2W, 2, gamma2, beta2, pvec[0:L2W, 7:8], "2")
        # only this core's shard continues past BN2
        lat = wp.tile([L2W, SH], MM_DT, name='dbg_lat', uniquify=False)
        nc.scalar.activation(lat, e2p[:, 0:SH], ACTF.Relu, bias=t2, scale=s2)

        # ---- gating MLP (shard only), transposed layout ----
        # L1 expert matmuls are interleaved into the PE stream between the
        # (tiny, dependency-chained) gating matmuls so the PE keeps busy
        # while DVE/ACT run the gating ELUs.
        y1 = pp.tile([SH, E * H], F32, tag="acc")  # 4 banks
        # lhsT rows 0:103 = xs^T, row 103 = 1.0 (host-set); ew1r row 103
        # holds eb1 -> each expert matmul already includes its bias.
        xsT = xTf[0 : DIN + 1, 0:SH]

        g1p = pp.tile([GH, SH], F32, tag="tr")
        _mm(nc, g1p, gw1T, lat)
        for j in (0, 1):
            _mm(nc, y1[:, 512 * j : 512 * (j + 1)], xsT, ew1f[0 : DIN + 1, 512 * j : 512 * (j + 1)])
        g1 = wp.tile([GH, SH], MM_DT)
        _elu(nc, wp, g1, g1p, bias_ap=gb1, tag="g")
        g2p = pp.tile([GH, SH], F32, tag="tr")
        _mm(nc, g2p, gw2T, g1)
        _mm(nc, y1[:, 1024:1536], xsT, ew1f[0 : DIN + 1, 1024:1536])
        g2 = wp.tile([GH, SH], MM_DT, name='dbg_g2', uniquify=False)
        _elu(nc, wp, g2, g2p, bias_ap=gb2, tag="g")
        g3p = pp.tile([E, SH], F32, tag="tr")
        _mm(nc, g3p, gw3T, g2)
        _mm(nc, y1[:, 1536:2048], xsT, ew1f[0 : DIN + 1, 1536:2048])
        g3 = wp.tile([E, SH], F32, name='dbg_g3', uniquify=False)
        nc.scalar.activation(g3, g3p, ACTF.Identity, bias=gb3)

        # softmax over experts: transpose to [batch, E].  No max-subtraction:
        # logits here are O(0.1), exp cannot overflow, and softmax is
        # shift-invariant so the result matches the reference to fp rounding.
        g3tp = pp.tile([SH, E], F32, tag="tr")
        nc.tensor.transpose(g3tp, g3, ident[0:E, 0:E])
        ex = wp.tile([SH, E], F32)
        ssum = wp.tile([SH, 1], F32)
        nc.scalar.activation(ex, g3tp, ACTF.Exp, accum_out=ssum)
        rs = wp.tile([SH, 1], F32)
        nc.vector.reciprocal(rs, ssum)
        bc = wp.tile([SH, E], F32, name='dbg_bc', uniquify=False)
        nc.vector.tensor_scalar(bc, ex, rs, None, ALU.mult)

        # ---- generator layer 1 blend (y1 matmuls already issued above) ----
        acc1 = wp.tile([SH, H], F32, name='dbg_acc1', uniquify=False)
        for j in range(4):
            for e in (2 * j, 2 * j + 1):
                if e == 0:
                    nc.vector.tensor_scalar(
                        acc1, y1[:, 0:H], bc[:, 0:1], None, ALU.mult
                    )
                else:
                    nc.vector.scalar_tensor_tensor(
                        acc1, y1[:, H * e : H * (e + 1)], bc[:, e : e + 1], acc1,
                        ALU.mult, ALU.add,
                    )
        h1 = wp.tile([SH, H], F32, name='dbg_h1', uniquify=False)
        _elu(nc, wp, h1, acc1, tag="h", on_gpsimd=True)

        # transpose h1 -> lhsT chunks
        h1T = []
        for c in range(2):
            trp = pp.tile([128, 128], F32, tag="tr")
            nc.tensor.transpose(trp, h1[:, 128 * c : 128 * (c + 1)], ident)
            t_ = wp.tile([128, 128], MM_DT, tag=f"h1T{c}")
            nc.scalar.copy(t_, trp)
            h1T.append(t_)

        # ---- generator layer 2 ----
        y2 = pp.tile([SH, E * H], F32, tag="acc")
        acc2 = wp.tile([SH, H], F32, name='dbg_acc2', uniquify=False)
        # j-outer / c-inner: psum bank j completes after its two chained
        # matmuls, so the DVE blend for experts 2j, 2j+1 overlaps the
        # remaining matmuls instead of trailing them.
        for j in range(4):
            for c in range(2):
                _mm(
                    nc,
                    y2[:, 512 * j : 512 * (j + 1)],
                    h1T[c],
                    ew2[:, 1024 * j + 512 * c : 1024 * j + 512 * (c + 1)],
                    start=(c == 0),
                    stop=False,
                )
            # K=1 rank-1 update adds the per-expert biases eb2 to bank j
            _mm(nc, y2[:, 512 * j : 512 * (j + 1)], ones_row,
                eb2f[:, 512 * j : 512 * (j + 1)], start=False, stop=True)
            for e in (2 * j, 2 * j + 1):
                if e == 0:
                    nc.vector.tensor_scalar(
                        acc2, y2[:, 0:H], bc[:, 0:1], None, ALU.mult
                    )
                else:
                    nc.vector.scalar_tensor_tensor(
                        acc2, y2[:, H * e : H * (e + 1)], bc[:, e : e + 1], acc2,
                        ALU.mult, ALU.add,
                    )
        h2 = wp.tile([SH, H], F32, name='dbg_h2', uniquify=False)
        _elu(nc, wp, h2, acc2, tag="h", on_gpsimd=True)

        h2T = []
        for c in range(2):
            trp = pp.tile([128, 128], F32, tag="tr")
            nc.tensor.transpose(trp, h2[:, 128 * c : 128 * (c + 1)], ident)
            t_ = wp.tile([128, 128], MM_DT, tag=f"h2T{c}")
            nc.scalar.copy(t_, trp)
            h2T.append(t_)

        # ---- generator layer 3 (all experts side by side, N = 8*51) ----
        oP = pp.tile([SH, E * DOUT], F32, tag="tr")
        for c in range(2):
            _mm(
                nc,
                oP,
                h2T[c],
                ew3[:, E * DOUT * c : E * DOUT * (c + 1)],
                start=(c == 0),
                stop=False,
            )
        _mm(nc, oP, ones_row, eb3f[:, 0 : E * DOUT], start=False, stop=True)
        osb = wp.tile([SH, DOUT], F32)
        nc.vector.tensor_scalar(osb, oP[:, 0:DOUT], bc[:, 0:1], None, ALU.mult)
        for e in range(1, E):
            nc.vector.scalar_tensor_tensor(
                osb, oP[:, DOUT * e : DOUT * (e + 1)], bc[:, e : e + 1], osb, ALU.mult, ALU.add
            )
        nc.sync.dma_start(out_ap, osb)


def build_program():
    nc = bacc.Bacc("TRN2", target_bir_lowering=False, debug=False, num_devices=NCORES)
    d = {}
    for name, shape, dt_ in [
        ("xT", (128, B), MM_DT),
        ("w1T", (128, L1W), MM_DT),
        ("pmat", (128, 168), MM_DT),
        ("pvec", (128, 9), F32),
        ("ew1r", (128, E * H), MM_DT),
        ("ew2r", (128, 2 * E * H), MM_DT),
        ("ew3r", (128, 2 * E * DOUT), MM_DT),
        ("eb2f", (1, E * H), MM_DT),
        ("eb3f", (1, E * DOUT + SH), MM_DT),
    ]:
        d[name] = nc.dram_tensor(name, list(shape), dt_, kind="ExternalInput").ap()
    out_ap = nc.dram_tensor("out", [SH, DOUT], F32, kind="ExternalOutput").ap()
    with tile.TileContext(nc) as tc:
        _build_kernel(tc, d, out_ap)
    nc.compile()
    return nc


def prep_in_maps(inputs):
    f = np.float32
    if MM_MODE == "bf16":
        import ml_dtypes

        mmf = ml_dtypes.bfloat16
    else:
        mmf = f

    def c(a):
        return np.ascontiguousarray(a, dtype=mmf)

    x = np.asarray(inputs["x"], dtype=f)

    def pad128(a):
        out = np.zeros((128, a.shape[1]), a.dtype)
        out[: a.shape[0]] = a
        return out

    w1T = pad128(c(np.asarray(inputs["w1"]).T))
    pmat = np.zeros((128, 168), mmf)
    pmat[0:64, 0:32] = np.asarray(inputs["w2"]).T       # [64, 32]
    pmat[0:32, 32:96] = np.asarray(inputs["gw1"]).T     # [32, 64]
    pmat[0:64, 96:160] = np.asarray(inputs["gw2"]).T    # [64, 64]
    pmat[0:64, 160:168] = np.asarray(inputs["gw3"]).T   # [64, 8]
    pvec = np.zeros((128, 9), f)
    pvec[:, 7] = EPS
    pvec[:, 8] = 1.0
    pvec[100:103, 8] = 100.0
    pvec[0:64, 0] = inputs["gamma1"]
    pvec[0:64, 1] = inputs["beta1"]
    pvec[0:32, 2] = inputs["gamma2"]
    pvec[0:32, 3] = inputs["beta2"]
    pvec[0:64, 4] = inputs["gb1"]
    pvec[0:64, 5] = inputs["gb2"]
    pvec[0:8, 6] = inputs["gb3"]
    ew1r = pad128(c(np.asarray(inputs["ew1"]).transpose(1, 0, 2).reshape(DIN, E * H)))
    ew1r[DIN, :] = np.asarray(inputs["eb1"], dtype=mmf).reshape(E * H)
    # [e, h, k] -> [p, j, c, i2, k] with e = 2j+i2, h = 128c+p: matmul (j, c)
    # reads the contiguous column block j*1024 + c*512.
    ew2r = c(
        np.asarray(inputs["ew2"])
        .reshape(4, 2, 2, 128, H)
        .transpose(3, 0, 2, 1, 4)
        .reshape(128, 2 * E * H)
    )
    ew3r = c(
        np.asarray(inputs["ew3"])
        .reshape(E, 2, 128, DOUT)
        .transpose(2, 1, 0, 3)
        .reshape(128, 2 * E * DOUT)
    )
    # eb2 laid out to match ew2r's (j, i2, k) column order; eb3 matches
    # ew3r's (e, o) order within one chunk.
    eb2f = c(np.asarray(inputs["eb2"]).reshape(1, E * H))
    eb3f = np.ones((1, E * DOUT + SH), mmf)
    eb3f[0, 0 : E * DOUT] = np.asarray(inputs["eb3"], dtype=mmf).reshape(E * DOUT)

    shared = {
        "w1T": w1T, "pmat": pmat, "pvec": pvec,
        "ew1r": ew1r, "ew2r": ew2r, "ew3r": ew3r, "eb2f": eb2f, "eb3f": eb3f,
    }
    in_maps = []
    for core in range(NCORES):
        xr = np.roll(x, -core * SH, axis=0)  # this core's shard -> rows 0:SH
        xt = pad128(c(xr.T))
        xt[DIN, :] = 1.0
        in_maps.append({**shared, "xT": xt})
    return in_maps


_prog = None


def _get_program():
    global _prog
    if _prog is None:
        _prog = build_program()
    return _prog


def kernel(**inputs) -> np.ndarray:
    nc = _get_program()
    in_maps = prep_in_maps(inputs)
    res = run_bass_kernel_spmd(nc, in_maps, core_ids=list(range(NCORES)))
    return np.concatenate(
        [np.asarray(res.results[cid]["out"]) for cid in range(NCORES)], axis=0
    )


# revision 24
# speedup vs baseline: 1.5182x; 1.1160x over previous
"""MoE-routing (MANN-style) network on 8 Trainium2 NeuronCores.

Strategy: pure data parallel. Each core receives the FULL batch
(transposed, batch-rotated so that "its" 128-sample shard sits at
columns 0:128) and computes:
  - encoder (Linear+BN+ReLU x2) over the full batch (BatchNorm in
    training mode needs full-batch statistics; the rotation makes the
    stats permutation-invariant and keeps the program identical across
    cores -- SPMD with per-core data only),
  - gating MLP + softmax for its shard only,
  - the expensive expert-blended 3-layer generator for its shard only.

All weights are pre-rearranged on the host into the exact SBUF layouts
the TensorEngine wants, so every DMA is a fully contiguous stream.
Expert blending sum_e bc[b,e] * (x @ W_e) is computed with per-expert
matmuls (the blend weight is a per-partition scalar in batch-on-
partitions layout, so it fuses into one scalar_tensor_tensor per
expert).  Matmuls run as float32r (full-rate fp32 path on TRN2).
"""

import numpy as np

import concourse.bacc as bacc
import concourse.tile as tile
from concourse import masks, mybir
from concourse.bass_utils import run_bass_kernel_spmd

F32 = mybir.dt.float32
F32R = mybir.dt.float32r
ALU = mybir.AluOpType
ACTF = mybir.ActivationFunctionType
AX = mybir.AxisListType

B, DIN, L1W, L2W, E, GH, H, DOUT = 1024, 103, 64, 32, 8, 64, 256, 51
NCORES = 8
SH = B // NCORES  # 128 samples per core
EPS = 1e-5

import os

# matmul operand mode:
#   "f32"  - plain fp32 (exact, 4 cycles/col on PE)
#   "f32r" - tfloat32 (full-rate 1 cycle/col at N>=256, ~10-bit mantissa)
#   "bf16" - bfloat16 operands (full rate, halves weight DMA traffic)
MM_MODE = os.environ.get("KERNEL_MM_MODE", "f32r")
BF16 = mybir.dt.bfloat16
MM_DT = {"f32": F32, "f32r": F32R, "bf16": BF16}[MM_MODE]


def _mm(nc, out, lhsT, rhs, start=True, stop=True):
    nc.tensor.matmul(out, lhsT, rhs, start=start, stop=stop)


def _elu(nc, pool, out, z, bias_ap=None, tag="", on_gpsimd=False):
    """out = elu(z + bias). elu(x) = relu(x) + exp(min(x,0)) - 1."""
    p, f = z.shape[0], z.free_size()
    mt = pool.tile([p, f], F32, tag=f"elu_m{tag}")
    rt = pool.tile([p, f], F32, tag=f"elu_r{tag}")
    et = pool.tile([p, f], F32, tag=f"elu_e{tag}")
    eng = nc.gpsimd if on_gpsimd else nc.vector  # gpsimd only for SBUF inputs
    if bias_ap is not None:
        eng.tensor_scalar(mt, z, bias_ap, 0.0, ALU.add, ALU.min)
        eng.tensor_scalar(rt, z, bias_ap, 0.0, ALU.add, ALU.max)
    else:
        eng.tensor_scalar(mt, z, 0.0, None, ALU.min)
        eng.tensor_scalar(rt, z, 0.0, None, ALU.max)
    nc.scalar.activation(et, mt, ACTF.Exp)
    # (exp(min) + (-1)) + relu
    nc.vector.scalar_tensor_tensor(out, et, -1.0, rt, ALU.add, ALU.add)


def _bn_apply_params(nc, pool, psum, width, nchunks, gamma_ap, beta_ap, eps_ap, tag):
    """Full-batch BN stats over psum [width, nchunks*512]; returns
    (scale, shift) APs [width, 1] s.t. bn(x) = scale*x + shift."""
    st = pool.tile([width, 6 * nchunks], F32, tag=f"bnst{tag}")
    for i in range(nchunks):
        nc.vector.bn_stats(st[:, 6 * i : 6 * (i + 1)], psum[:, 512 * i : 512 * (i + 1)])
    mv = pool.tile([width, 2], F32, tag=f"bnmv{tag}")
    nc.vector.bn_aggr(mv, st)
    # rsqrt(v+eps) = exp(-0.5*ln(v+eps)): Ln+Exp live in the same ACT
    # table set as Relu/Copy/Identity, so no 1.3us table_sel switches.
    lv = pool.tile([width, 1], F32, tag=f"bnlv{tag}")
    nc.scalar.activation(lv, mv[:, 1:2], ACTF.Ln, bias=eps_ap)
    rstd = pool.tile([width, 1], F32, tag=f"bnrs{tag}")
    nc.scalar.activation(rstd, lv, ACTF.Exp, scale=-0.5)
    s = pool.tile([width, 1], F32, tag=f"bns{tag}")
    nc.vector.tensor_tensor(s, gamma_ap, rstd, ALU.mult)
    sm = pool.tile([width, 1], F32, tag=f"bnsm{tag}")
    nc.vector.tensor_tensor(sm, mv[:, 0:1], s, ALU.mult)
    t = pool.tile([width, 1], F32, tag=f"bnt{tag}")
    nc.vector.tensor_tensor(t, beta_ap, sm, ALU.subtract)  # beta - s*m
    return s, t


def _build_kernel(tc, d, out_ap):
    nc = tc.nc
    with (
        tc.tile_pool(name="const", bufs=1) as cp,
        tc.tile_pool(name="work", bufs=1) as wp,
        tc.tile_pool(name="psum", bufs=1, space="PSUM") as pp,
    ):
        # ---- constant loads (contiguous DMAs) ----
        ident = cp.tile([128, 128], F32)
        masks.make_identity(nc, ident)

        # All big tensors are padded to 128 partitions and loaded with ONE
        # dense dma_start each: HBM->SBUF transfers only fan out across the
        # 16 SDMA engines when the destination spans all 128 partitions --
        # sub-128-partition transfers drain through a single engine at
        # ~25 GB/s instead of ~340 GB/s.  Two HWDGE queues (sync + scalar)
        # drain in parallel.
        xTf = cp.tile([128, B], MM_DT)
        nc.sync.dma_start(xTf[:, 0:512], d["xT"][:, 0:512])
        nc.sync.dma_start(xTf[:, 512:B], d["xT"][:, 512:B])
        xT = xTf[0:DIN, :]
        pvec = cp.tile([128, 9], F32)
        nc.scalar.dma_start(pvec, d["pvec"])
        ew1f = cp.tile([128, E * H], MM_DT)
        nc.scalar.dma_start(ew1f, d["ew1r"])
        ew1 = ew1f[0:DIN, :]
        w1Tf = cp.tile([128, L1W], MM_DT)
        nc.scalar.dma_start(w1Tf, d["w1T"])
        w1T = w1Tf[0:DIN, :]
        pmatf = cp.tile([128, 168], MM_DT)
        nc.scalar.dma_start(pmatf, d["pmat"])
        pmat = pmatf[0:64, :]
        ew2 = cp.tile([128, 2 * E * H], MM_DT)
        for q in range(4):
            nc.sync.dma_start(
                ew2[:, 1024 * q : 1024 * (q + 1)], d["ew2r"][:, 1024 * q : 1024 * (q + 1)]
            )
        # late loads (needed from L2 on) are declared here but their DMA
        # triggers are emitted after the encoder so they do not occupy the
        # ACT engine while it has critical-path work
        eb2f = cp.tile([1, E * H], MM_DT)
        eb3f = cp.tile([1, E * DOUT + SH], MM_DT)
        ones_row = eb3f[:, E * DOUT : E * DOUT + SH]
        ew3 = cp.tile([128, 2 * E * DOUT], MM_DT)

        # param views
        w2T = pmat[:, 0:32]          # [64, 32]
        gw1T = pmat[0:32, 32:96]     # [32, 64]
        gw2T = pmat[:, 96:160]       # [64, 64]
        gw3T = pmat[:, 160:168]      # [64, 8]
        gamma1, beta1 = pvec[0:L1W, 0:1], pvec[0:L1W, 1:2]
        gamma2, beta2 = pvec[0:L2W, 2:3], pvec[0:L2W, 3:4]
        gb1, gb2, gb3 = pvec[0:GH, 4:5], pvec[0:GH, 5:6], pvec[0:E, 6:7]

        # scale last 3 input features by 100 (reference: x[:,100:103] *= 100)
        # partition offsets must be multiples of 32 -> scale rows 96:103 by a
        # per-partition vector (1 except rows 100..102 = 100); per column
        # chunk so encoder matmul j0 can start as soon as chunk 0 landed
        nc.vector.tensor_scalar(xT[96:DIN, 0:512], xT[96:DIN, 0:512], pvec[96:DIN, 8:9], None, ALU.mult)
        nc.vector.tensor_scalar(xT[96:DIN, 512:B], xT[96:DIN, 512:B], pvec[96:DIN, 8:9], None, ALU.mult)

        # ---- encoder, full batch, transposed layout [feat, batch] ----
        e1p = pp.tile([L1W, B], F32, tag="enc")  # 2 banks
        for j in range(2):
            _mm(nc, e1p[:, 512 * j : 512 * (j + 1)], w1T, xT[:, 512 * j : 512 * (j + 1)])
        s1, t1 = _bn_apply_params(nc, wp, e1p, L1W, 2, gamma1, beta1, pvec[0:L1W, 7:8], "1")
        e1 = wp.tile([L1W, B], MM_DT, name='dbg_e1', uniquify=False)
        nc.scalar.activation(e1, e1p, ACTF.Relu, bias=t1, scale=s1)
        nc.scalar.dma_start(eb2f, d["eb2f"])
        nc.scalar.dma_start(eb3f, d["eb3f"])
        nc.scalar.dma_start(ew3, d["ew3r"])

        e2p = pp.tile([L2W, B], F32, tag="enc")
        for j in range(2):
            _mm(nc, e2p[:, 512 * j : 512 * (j + 1)], w2T, e1[:, 512 * j : 512 * (j + 1)])
        s2, t2 = _bn_apply_params(nc, wp, e2p, L2W, 2, gamma2, beta2, pvec[0:L2W, 7:8], "2")
        # only this core's shard continues past BN2
        lat = wp.tile([L2W, SH], MM_DT, name='dbg_lat', uniquify=False)
        nc.scalar.activation(lat, e2p[:, 0:SH], ACTF.Relu, bias=t2, scale=s2)

        # ---- gating MLP (shard only), transposed layout ----
        # L1 expert matmuls are interleaved into the PE stream between the
        # (tiny, dependency-chained) gating matmuls so the PE keeps busy
        # while DVE/ACT run the gating ELUs.
        # one PSUM tile per 512-wide bank so a bank's blend (DVE read) never
        # blocks the next bank's matmuls via tile-level WAR dependencies
        y1 = [pp.tile([SH, 512], F32, tag="yb", name=f"y1b{j}") for j in range(4)]
        # lhsT rows 0:103 = xs^T, row 103 = 1.0 (host-set); ew1r row 103
        # holds eb1 -> each expert matmul already includes its bias.
        xsT = xTf[0 : DIN + 1, 0:SH]

        g1p = pp.tile([GH, SH], F32, tag="tr")
        _mm(nc, g1p, gw1T, lat)
        for j in (0, 1):
            _mm(nc, y1[j], xsT, ew1f[0 : DIN + 1, 512 * j : 512 * (j + 1)])
        g1 = wp.tile([GH, SH], MM_DT)
        _elu(nc, wp, g1, g1p, bias_ap=gb1, tag="g")
        g2p = pp.tile([GH, SH], F32, tag="tr")
        _mm(nc, g2p, gw2T, g1)
        _mm(nc, y1[2], xsT, ew1f[0 : DIN + 1, 1024:1536])
        g2 = wp.tile([GH, SH], MM_DT, name='dbg_g2', uniquify=False)
        _elu(nc, wp, g2, g2p, bias_ap=gb2, tag="g")
        g3p = pp.tile([E, SH], F32, tag="tr")
        _mm(nc, g3p, gw3T, g2)
        _mm(nc, y1[3], xsT, ew1f[0 : DIN + 1, 1536:2048])
        g3 = wp.tile([E, SH], F32, name='dbg_g3', uniquify=False)
        nc.scalar.activation(g3, g3p, ACTF.Identity, bias=gb3)

        # softmax over experts: transpose to [batch, E].  No max-subtraction:
        # logits here are O(0.1), exp cannot overflow, and softmax is
        # shift-invariant so the result matches the reference to fp rounding.
        g3tp = pp.tile([SH, E], F32, tag="tr")
        nc.tensor.transpose(g3tp, g3, ident[0:E, 0:E])
        ex = wp.tile([SH, E], F32)
        ssum = wp.tile([SH, 1], F32)
        nc.scalar.activation(ex, g3tp, ACTF.Exp, accum_out=ssum)
        rs = wp.tile([SH, 1], F32)
        nc.vector.reciprocal(rs, ssum)
        bc = wp.tile([SH, E], F32, name='dbg_bc', uniquify=False)
        nc.vector.tensor_scalar(bc, ex, rs, None, ALU.mult)

        # ---- generator layer 1 blend (y1 matmuls already issued above) ----
        acc1 = wp.tile([SH, H], F32, name='dbg_acc1', uniquify=False)
        for j in range(4):
            for i2 in (0, 1):
                e = 2 * j + i2
                ysl = y1[j][:, H * i2 : H * (i2 + 1)]
                if e == 0:
                    nc.vector.tensor_scalar(acc1, ysl, bc[:, 0:1], None, ALU.mult)
                else:
                    nc.vector.scalar_tensor_tensor(
                        acc1, ysl, bc[:, e : e + 1], acc1, ALU.mult, ALU.add,
                    )
        h1 = wp.tile([SH, H], F32, name='dbg_h1', uniquify=False)
        _elu(nc, wp, h1, acc1, tag="h")

        # transpose h1 -> lhsT chunks
        h1T = []
        for c in range(2):
            trp = pp.tile([128, 128], F32, tag="tr")
            nc.tensor.transpose(trp, h1[:, 128 * c : 128 * (c + 1)], ident)
            t_ = wp.tile([128, 128], MM_DT, tag=f"h1T{c}")
            nc.scalar.copy(t_, trp)
            h1T.append(t_)

        # ---- generator layer 2 ----
        acc2 = wp.tile([SH, H], F32, name='dbg_acc2', uniquify=False)
        # j-outer / c-inner: bank j completes after its three chained
        # matmuls, so the DVE blend for experts 2j, 2j+1 overlaps the
        # remaining matmuls instead of trailing them.
        for j in range(4):
            y2j = pp.tile([SH, 512], F32, tag="yb", name=f"y2b{j}")
            for c in range(2):
                _mm(
                    nc,
                    y2j,
                    h1T[c],
                    ew2[:, 1024 * j + 512 * c : 1024 * j + 512 * (c + 1)],
                    start=(c == 0),
                    stop=False,
                )
            # K=1 rank-1 update adds the per-expert biases eb2 to bank j
            _mm(nc, y2j, ones_row, eb2f[:, 512 * j : 512 * (j + 1)],
                start=False, stop=True)
            for i2 in (0, 1):
                e = 2 * j + i2
                ysl = y2j[:, H * i2 : H * (i2 + 1)]
                if e == 0:
                    nc.vector.tensor_scalar(acc2, ysl, bc[:, 0:1], None, ALU.mult)
                else:
                    nc.vector.scalar_tensor_tensor(
                        acc2, ysl, bc[:, e : e + 1], acc2, ALU.mult, ALU.add,
                    )
        h2 = wp.tile([SH, H], F32, name='dbg_h2', uniquify=False)
        _elu(nc, wp, h2, acc2, tag="h")

        h2T = []
        for c in range(2):
            trp = pp.tile([128, 128], F32, tag="tr")
            nc.tensor.transpose(trp, h2[:, 128 * c : 128 * (c + 1)], ident)
            t_ = wp.tile([128, 128], MM_DT, tag=f"h2T{c}")
            nc.scalar.copy(t_, trp)
            h2T.append(t_)

        # ---- generator layer 3 (all experts side by side, N = 8*51) ----
        oP = pp.tile([SH, E * DOUT], F32, tag="tr")
        for c in range(2):
            _mm(
                nc,
                oP,
                h2T[c],
                ew3[:, E * DOUT * c : E * DOUT * (c + 1)],
                start=(c == 0),
                stop=False,
            )
        _mm(nc, oP, ones_row, eb3f[:, 0 : E * DOUT], start=False, stop=True)
        osb = wp.tile([SH, DOUT], F32)
        nc.vector.tensor_scalar(osb, oP[:, 0:DOUT], bc[:, 0:1], None, ALU.mult)
        for e in range(1, E):
            nc.vector.scalar_tensor_tensor(
                osb, oP[:, DOUT * e : DOUT * (e + 1)], bc[:, e : e + 1], osb, ALU.mult, ALU.add
            )
        nc.sync.dma_start(out_ap, osb)


def build_program():
    nc = bacc.Bacc("TRN2", target_bir_lowering=False, debug=False, num_devices=NCORES)
    d = {}
    for name, shape, dt_ in [
        ("xT", (128, B), MM_DT),
        ("w1T", (128, L1W), MM_DT),
        ("pmat", (128, 168), MM_DT),
        ("pvec", (128, 9), F32),
        ("ew1r", (128, E * H), MM_DT),
        ("ew2r", (128, 2 * E * H), MM_DT),
        ("ew3r", (128, 2 * E * DOUT), MM_DT),
        ("eb2f", (1, E * H), MM_DT),
        ("eb3f", (1, E * DOUT + SH), MM_DT),
    ]:
        d[name] = nc.dram_tensor(name, list(shape), dt_, kind="ExternalInput").ap()
    out_ap = nc.dram_tensor("out", [SH, DOUT], F32, kind="ExternalOutput").ap()
    with tile.TileContext(nc) as tc:
        _build_kernel(tc, d, out_ap)
    nc.compile()
    return nc


def prep_in_maps(inputs):
    f = np.float32
    if MM_MODE == "bf16":
        import ml_dtypes

        mmf = ml_dtypes.bfloat16
    else:
        mmf = f

    def c(a):
        return np.ascontiguousarray(a, dtype=mmf)

    x = np.asarray(inputs["x"], dtype=f)

    def pad128(a):
        out = np.zeros((128, a.shape[1]), a.dtype)
        out[: a.shape[0]] = a
        return out

    w1T = pad128(c(np.asarray(inputs["w1"]).T))
    pmat = np.zeros((128, 168), mmf)
    pmat[0:64, 0:32] = np.asarray(inputs["w2"]).T       # [64, 32]
    pmat[0:32, 32:96] = np.asarray(inputs["gw1"]).T     # [32, 64]
    pmat[0:64, 96:160] = np.asarray(inputs["gw2"]).T    # [64, 64]
    pmat[0:64, 160:168] = np.asarray(inputs["gw3"]).T   # [64, 8]
    pvec = np.zeros((128, 9), f)
    pvec[:, 7] = EPS
    pvec[:, 8] = 1.0
    pvec[100:103, 8] = 100.0
    pvec[0:64, 0] = inputs["gamma1"]
    pvec[0:64, 1] = inputs["beta1"]
    pvec[0:32, 2] = inputs["gamma2"]
    pvec[0:32, 3] = inputs["beta2"]
    pvec[0:64, 4] = inputs["gb1"]
    pvec[0:64, 5] = inputs["gb2"]
    pvec[0:8, 6] = inputs["gb3"]
    ew1r = pad128(c(np.asarray(inputs["ew1"]).transpose(1, 0, 2).reshape(DIN, E * H)))
    ew1r[DIN, :] = np.asarray(inputs["eb1"], dtype=mmf).reshape(E * H)
    # [e, h, k] -> [p, j, c, i2, k] with e = 2j+i2, h = 128c+p: matmul (j, c)
    # reads the contiguous column block j*1024 + c*512.
    ew2r = c(
        np.asarray(inputs["ew2"])
        .reshape(4, 2, 2, 128, H)
        .transpose(3, 0, 2, 1, 4)
        .reshape(128, 2 * E * H)
    )
    ew3r = c(
        np.asarray(inputs["ew3"])
        .reshape(E, 2, 128, DOUT)
        .transpose(2, 1, 0, 3)
        .reshape(128, 2 * E * DOUT)
    )
    # eb2 laid out to match ew2r's (j, i2, k) column order; eb3 matches
    # ew3r's (e, o) order within one chunk.
    eb2f = c(np.asarray(inputs["eb2"]).reshape(1, E * H))
    eb3f = np.ones((1, E * DOUT + SH), mmf)
    eb3f[0, 0 : E * DOUT] = np.asarray(inputs["eb3"], dtype=mmf).reshape(E * DOUT)

    shared = {
        "w1T": w1T, "pmat": pmat, "pvec": pvec,
        "ew1r": ew1r, "ew2r": ew2r, "ew3r": ew3r, "eb2f": eb2f, "eb3f": eb3f,
    }
    in_maps = []
    for core in range(NCORES):
        xr = np.roll(x, -core * SH, axis=0)  # this core's shard -> rows 0:SH
        xt = pad128(c(xr.T))
        xt[DIN, :] = 1.0
        in_maps.append({**shared, "xT": xt})
    return in_maps


_prog = None


def _get_program():
    global _prog
    if _prog is None:
        _prog = build_program()
    return _prog


def kernel(**inputs) -> np.ndarray:
    nc = _get_program()
    in_maps = prep_in_maps(inputs)
    res = run_bass_kernel_spmd(nc, in_maps, core_ids=list(range(NCORES)))
    return np.concatenate(
        [np.asarray(res.results[cid]["out"]) for cid in range(NCORES)], axis=0
    )


# revision 26
# speedup vs baseline: 1.6417x; 1.0814x over previous
"""MoE-routing (MANN-style) network on 8 Trainium2 NeuronCores.

Strategy: pure data parallel. Each core receives the FULL batch
(transposed, batch-rotated so that "its" 128-sample shard sits at
columns 0:128) and computes:
  - encoder (Linear+BN+ReLU x2) over the full batch (BatchNorm in
    training mode needs full-batch statistics; the rotation makes the
    stats permutation-invariant and keeps the program identical across
    cores -- SPMD with per-core data only),
  - gating MLP + softmax for its shard only,
  - the expensive expert-blended 3-layer generator for its shard only.

All weights are pre-rearranged on the host into the exact SBUF layouts
the TensorEngine wants, so every DMA is a fully contiguous stream.
Expert blending sum_e bc[b,e] * (x @ W_e) is computed with per-expert
matmuls (the blend weight is a per-partition scalar in batch-on-
partitions layout, so it fuses into one scalar_tensor_tensor per
expert).  Matmuls run as float32r (full-rate fp32 path on TRN2).
"""

import numpy as np

import concourse.bacc as bacc
import concourse.tile as tile
from concourse import masks, mybir
from concourse.bass_utils import run_bass_kernel_spmd

F32 = mybir.dt.float32
F32R = mybir.dt.float32r
ALU = mybir.AluOpType
ACTF = mybir.ActivationFunctionType
AX = mybir.AxisListType

B, DIN, L1W, L2W, E, GH, H, DOUT = 1024, 103, 64, 32, 8, 64, 256, 51
NCORES = 8
SH = B // NCORES  # 128 samples per core
EPS = 1e-5

import os

# matmul operand mode:
#   "f32"  - plain fp32 (exact, 4 cycles/col on PE)
#   "f32r" - tfloat32 (full-rate 1 cycle/col at N>=256, ~10-bit mantissa)
#   "bf16" - bfloat16 operands (full rate, halves weight DMA traffic)
MM_MODE = os.environ.get("KERNEL_MM_MODE", "f32r")
BF16 = mybir.dt.bfloat16
MM_DT = {"f32": F32, "f32r": F32R, "bf16": BF16}[MM_MODE]


def _mm(nc, out, lhsT, rhs, start=True, stop=True):
    nc.tensor.matmul(out, lhsT, rhs, start=start, stop=stop)


def _elu(nc, pool, out, z, bias_ap=None, tag="", on_gpsimd=False):
    """out = elu(z + bias). elu(x) = relu(x) + exp(min(x,0)) - 1."""
    p, f = z.shape[0], z.free_size()
    mt = pool.tile([p, f], F32, tag=f"elu_m{tag}")
    rt = pool.tile([p, f], F32, tag=f"elu_r{tag}")
    et = pool.tile([p, f], F32, tag=f"elu_e{tag}")
    eng = nc.gpsimd if on_gpsimd else nc.vector  # gpsimd only for SBUF inputs
    if bias_ap is not None:
        eng.tensor_scalar(mt, z, bias_ap, 0.0, ALU.add, ALU.min)
        eng.tensor_scalar(rt, z, bias_ap, 0.0, ALU.add, ALU.max)
    else:
        eng.tensor_scalar(mt, z, 0.0, None, ALU.min)
        eng.tensor_scalar(rt, z, 0.0, None, ALU.max)
    nc.scalar.activation(et, mt, ACTF.Exp)
    # (exp(min) + (-1)) + relu
    nc.vector.scalar_tensor_tensor(out, et, -1.0, rt, ALU.add, ALU.add)


def _bn_apply_params(nc, pool, psum, width, nchunks, gamma_ap, beta_ap, eps_ap, tag):
    """Full-batch BN stats over psum [width, nchunks*512]; returns
    (scale, shift) APs [width, 1] s.t. bn(x) = scale*x + shift."""
    st = pool.tile([width, 6 * nchunks], F32, tag=f"bnst{tag}")
    for i in range(nchunks):
        nc.vector.bn_stats(st[:, 6 * i : 6 * (i + 1)], psum[:, 512 * i : 512 * (i + 1)])
    mv = pool.tile([width, 2], F32, tag=f"bnmv{tag}")
    nc.vector.bn_aggr(mv, st)
    sd = pool.tile([width, 1], F32, tag=f"bnsd{tag}")
    nc.scalar.activation(sd, mv[:, 1:2], ACTF.Sqrt, bias=eps_ap)  # std
    rstd = pool.tile([width, 1], F32, tag=f"bnrs{tag}")
    nc.vector.reciprocal(rstd, sd)
    s = pool.tile([width, 1], F32, tag=f"bns{tag}")
    nc.vector.tensor_tensor(s, gamma_ap, rstd, ALU.mult)
    sm = pool.tile([width, 1], F32, tag=f"bnsm{tag}")
    nc.vector.tensor_tensor(sm, mv[:, 0:1], s, ALU.mult)
    t = pool.tile([width, 1], F32, tag=f"bnt{tag}")
    nc.vector.tensor_tensor(t, beta_ap, sm, ALU.subtract)  # beta - s*m
    return s, t


def _build_kernel(tc, d, out_ap):
    nc = tc.nc
    with (
        tc.tile_pool(name="const", bufs=1) as cp,
        tc.tile_pool(name="work", bufs=1) as wp,
        tc.tile_pool(name="psum", bufs=1, space="PSUM") as pp,
    ):
        # ---- constant loads (contiguous DMAs) ----
        ident = cp.tile([128, 128], F32)
        masks.make_identity(nc, ident)

        # All big tensors are padded to 128 partitions and loaded with ONE
        # dense dma_start each: HBM->SBUF transfers only fan out across the
        # 16 SDMA engines when the destination spans all 128 partitions --
        # sub-128-partition transfers drain through a single engine at
        # ~25 GB/s instead of ~340 GB/s.  Two HWDGE queues (sync + scalar)
        # drain in parallel.
        xTf = cp.tile([128, B], MM_DT)
        nc.sync.dma_start(xTf, d["xT"])
        xT = xTf[0:DIN, :]
        pvec = cp.tile([128, 9], F32)
        nc.scalar.dma_start(pvec, d["pvec"])
        ew1f = cp.tile([128, E * H], MM_DT)
        nc.scalar.dma_start(ew1f, d["ew1r"])
        ew1 = ew1f[0:DIN, :]
        w1Tf = cp.tile([128, L1W], MM_DT)
        nc.scalar.dma_start(w1Tf, d["w1T"])
        w1T = w1Tf[0:DIN, :]
        pmatf = cp.tile([128, 168], MM_DT)
        nc.scalar.dma_start(pmatf, d["pmat"])
        pmat = pmatf[0:64, :]
        ew2 = cp.tile([128, 2 * E * H], MM_DT)
        for q in range(4):
            nc.sync.dma_start(
                ew2[:, 1024 * q : 1024 * (q + 1)], d["ew2r"][:, 1024 * q : 1024 * (q + 1)]
            )
        # late loads (needed from L2 on) are declared here but their DMA
        # triggers are emitted after the encoder so they do not occupy the
        # ACT engine while it has critical-path work
        ebp = cp.tile([E, 308], MM_DT)
        ew3 = cp.tile([128, 2 * E * DOUT], MM_DT)

        # param views
        w2T = pmat[:, 0:32]          # [64, 32]
        gw1T = pmat[0:32, 32:96]     # [32, 64]
        gw2T = pmat[:, 96:160]       # [64, 64]
        gw3T = pmat[:, 160:168]      # [64, 8]
        gamma1, beta1 = pvec[0:L1W, 0:1], pvec[0:L1W, 1:2]
        gamma2, beta2 = pvec[0:L2W, 2:3], pvec[0:L2W, 3:4]
        gb1, gb2, gb3 = pvec[0:GH, 4:5], pvec[0:GH, 5:6], pvec[0:E, 6:7]

        # scale last 3 input features by 100 (reference: x[:,100:103] *= 100)
        # partition offsets must be multiples of 32 -> scale rows 96:103 by a
        # per-partition vector (1 except rows 100..102 = 100); per column
        # chunk so encoder matmul j0 can start as soon as chunk 0 landed
        nc.vector.tensor_scalar(xT[96:DIN, :], xT[96:DIN, :], pvec[96:DIN, 8:9], None, ALU.mult)

        # ---- encoder, full batch, transposed layout [feat, batch] ----
        e1p = pp.tile([L1W, B], F32, tag="enc")  # 2 banks
        for j in range(2):
            _mm(nc, e1p[:, 512 * j : 512 * (j + 1)], w1T, xT[:, 512 * j : 512 * (j + 1)])
        s1, t1 = _bn_apply_params(nc, wp, e1p, L1W, 2, gamma1, beta1, pvec[0:L1W, 7:8], "1")
        e1 = wp.tile([L1W, B], MM_DT, name='dbg_e1', uniquify=False)
        e1a = wp.tile([L1W, B], F32, tag="e1a")
        nc.vector.tensor_scalar(e1a, e1p, s1, t1, ALU.mult, ALU.add)
        nc.vector.tensor_scalar(e1, e1a, 0.0, None, ALU.max)
        nc.scalar.dma_start(ebp, d["ebp"])
        nc.scalar.dma_start(ew3, d["ew3r"])

        e2p = pp.tile([L2W, B], F32, tag="enc")
        for j in range(2):
            _mm(nc, e2p[:, 512 * j : 512 * (j + 1)], w2T, e1[:, 512 * j : 512 * (j + 1)])
        s2, t2 = _bn_apply_params(nc, wp, e2p, L2W, 2, gamma2, beta2, pvec[0:L2W, 7:8], "2")
        # only this core's shard continues past BN2
        lat = wp.tile([L2W, SH], MM_DT, name='dbg_lat', uniquify=False)
        lata = wp.tile([L2W, SH], F32, tag="lata")
        nc.vector.tensor_scalar(lata, e2p[:, 0:SH], s2, t2, ALU.mult, ALU.add)
        nc.vector.tensor_scalar(lat, lata, 0.0, None, ALU.max)

        # ---- gating MLP (shard only), transposed layout ----
        # L1 expert matmuls are interleaved into the PE stream between the
        # (tiny, dependency-chained) gating matmuls so the PE keeps busy
        # while DVE/ACT run the gating ELUs.
        # one PSUM tile per 512-wide bank so a bank's blend (DVE read) never
        # blocks the next bank's matmuls via tile-level WAR dependencies
        y1 = [pp.tile([SH, 512], F32, tag="yb", name=f"y1b{j}") for j in range(4)]
        # lhsT rows 0:103 = xs^T, row 103 = 1.0 (host-set); ew1r row 103
        # holds eb1 -> each expert matmul already includes its bias.
        xsT = xTf[0 : DIN + 1, 0:SH]

        g1p = pp.tile([GH, SH], F32, tag="tr")
        _mm(nc, g1p, gw1T, lat)
        for j in (0, 1):
            _mm(nc, y1[j], xsT, ew1f[0 : DIN + 1, 512 * j : 512 * (j + 1)])
        g1 = wp.tile([GH, SH], MM_DT)
        _elu(nc, wp, g1, g1p, bias_ap=gb1, tag="g")
        g2p = pp.tile([GH, SH], F32, tag="tr")
        _mm(nc, g2p, gw2T, g1)
        _mm(nc, y1[2], xsT, ew1f[0 : DIN + 1, 1024:1536])
        g2 = wp.tile([GH, SH], MM_DT, name='dbg_g2', uniquify=False)
        _elu(nc, wp, g2, g2p, bias_ap=gb2, tag="g")
        g3p = pp.tile([E, SH], F32, tag="tr")
        _mm(nc, g3p, gw3T, g2)
        _mm(nc, y1[3], xsT, ew1f[0 : DIN + 1, 1536:2048])
        g3 = wp.tile([E, SH], F32, name='dbg_g3', uniquify=False)
        nc.vector.tensor_scalar(g3, g3p, gb3, None, ALU.add)

        # softmax over experts: transpose to [batch, E].  No max-subtraction:
        # logits here are O(0.1), exp cannot overflow, and softmax is
        # shift-invariant so the result matches the reference to fp rounding.
        g3tp = pp.tile([SH, E], F32, tag="tr")
        nc.tensor.transpose(g3tp, g3, ident[0:E, 0:E])
        ex = wp.tile([SH, E], F32)
        ssum = wp.tile([SH, 1], F32)
        nc.scalar.activation(ex, g3tp, ACTF.Exp, accum_out=ssum)
        rs = wp.tile([SH, 1], F32)
        nc.vector.reciprocal(rs, ssum)
        bc = wp.tile([SH, E], F32, name='dbg_bc', uniquify=False)
        nc.vector.tensor_scalar(bc, ex, rs, None, ALU.mult)
        bctp = pp.tile([E, SH], F32, tag="tr")
        nc.tensor.transpose(bctp, bc, ident)
        bcT = wp.tile([E, SH], MM_DT)
        nc.scalar.copy(bcT, bctp)
        # bias2|bias3 = bc @ [eb2 | eb3] in one matmul (N=308, even)
        b23p = pp.tile([SH, 308], F32, tag="tr")
        _mm(nc, b23p, bcT, ebp)
        b23s = wp.tile([SH, 308], F32)
        nc.scalar.copy(b23s, b23p)

        # ---- generator layer 1 blend (y1 matmuls already issued above) ----
        acc1 = wp.tile([SH, H], F32, name='dbg_acc1', uniquify=False)
        for j in range(4):
            for i2 in (0, 1):
                e = 2 * j + i2
                ysl = y1[j][:, H * i2 : H * (i2 + 1)]
                if e == 0:
                    nc.vector.tensor_scalar(acc1, ysl, bc[:, 0:1], None, ALU.mult)
                else:
                    nc.vector.scalar_tensor_tensor(
                        acc1, ysl, bc[:, e : e + 1], acc1, ALU.mult, ALU.add,
                    )
        h1 = wp.tile([SH, H], F32, name='dbg_h1', uniquify=False)
        _elu(nc, wp, h1, acc1, tag="h")

        # transpose h1 -> lhsT chunks
        h1T = []
        for c in range(2):
            trp = pp.tile([128, 128], F32, tag="tr")
            nc.tensor.transpose(trp, h1[:, 128 * c : 128 * (c + 1)], ident)
            t_ = wp.tile([128, 128], MM_DT, tag=f"h1T{c}")
            nc.scalar.copy(t_, trp)
            h1T.append(t_)

        # ---- generator layer 2 ----
        acc2 = wp.tile([SH, H], F32, name='dbg_acc2', uniquify=False)
        # j-outer / c-inner matmuls; blend emission lags one bank group so
        # the PE never waits on the DVE blend chain.
        y2b = []

        def _blend2(j):
            for i2 in (0, 1):
                e = 2 * j + i2
                ysl = y2b[j][:, H * i2 : H * (i2 + 1)]
                if e == 0:
                    nc.vector.scalar_tensor_tensor(
                        acc2, ysl, bc[:, 0:1], b23s[:, 0:H], ALU.mult, ALU.add,
                    )
                else:
                    nc.vector.scalar_tensor_tensor(
                        acc2, ysl, bc[:, e : e + 1], acc2, ALU.mult, ALU.add,
                    )

        for j in range(4):
            y2j = pp.tile([SH, 512], F32, tag="yb", name=f"y2b{j}")
            y2b.append(y2j)
            for c in range(2):
                _mm(
                    nc,
                    y2j,
                    h1T[c],
                    ew2[:, 1024 * j + 512 * c : 1024 * j + 512 * (c + 1)],
                    start=(c == 0),
                    stop=(c == 1),
                )
            if j >= 1:
                _blend2(j - 1)
        _blend2(3)
        h2 = wp.tile([SH, H], F32, name='dbg_h2', uniquify=False)
        _elu(nc, wp, h2, acc2, tag="h")

        h2T = []
        for c in range(2):
            trp = pp.tile([128, 128], F32, tag="tr")
            nc.tensor.transpose(trp, h2[:, 128 * c : 128 * (c + 1)], ident)
            t_ = wp.tile([128, 128], MM_DT, tag=f"h2T{c}")
            nc.scalar.copy(t_, trp)
            h2T.append(t_)

        # ---- generator layer 3 (all experts side by side, N = 8*51) ----
        oP = pp.tile([SH, E * DOUT], F32, tag="tr")
        for c in range(2):
            _mm(
                nc,
                oP,
                h2T[c],
                ew3[:, E * DOUT * c : E * DOUT * (c + 1)],
                start=(c == 0),
                stop=(c == 1),
            )
        osb = wp.tile([SH, DOUT], F32)
        nc.vector.scalar_tensor_tensor(
            osb, oP[:, 0:DOUT], bc[:, 0:1], b23s[:, H : H + DOUT], ALU.mult, ALU.add
        )
        for e in range(1, E):
            nc.vector.scalar_tensor_tensor(
                osb, oP[:, DOUT * e : DOUT * (e + 1)], bc[:, e : e + 1], osb, ALU.mult, ALU.add
            )
        nc.sync.dma_start(out_ap, osb)


def build_program():
    nc = bacc.Bacc("TRN2", target_bir_lowering=False, debug=False, num_devices=NCORES)
    d = {}
    for name, shape, dt_ in [
        ("xT", (128, B), MM_DT),
        ("w1T", (128, L1W), MM_DT),
        ("pmat", (128, 168), MM_DT),
        ("pvec", (128, 9), F32),
        ("ew1r", (128, E * H), MM_DT),
        ("ew2r", (128, 2 * E * H), MM_DT),
        ("ew3r", (128, 2 * E * DOUT), MM_DT),
        ("ebp", (E, 308), MM_DT),
    ]:
        d[name] = nc.dram_tensor(name, list(shape), dt_, kind="ExternalInput").ap()
    out_ap = nc.dram_tensor("out", [SH, DOUT], F32, kind="ExternalOutput").ap()
    with tile.TileContext(nc) as tc:
        _build_kernel(tc, d, out_ap)
    nc.compile()
    return nc


def prep_in_maps(inputs):
    f = np.float32
    if MM_MODE == "bf16":
        import ml_dtypes

        mmf = ml_dtypes.bfloat16
    else:
        mmf = f

    def c(a):
        return np.ascontiguousarray(a, dtype=mmf)

    x = np.asarray(inputs["x"], dtype=f)

    def pad128(a):
        out = np.zeros((128, a.shape[1]), a.dtype)
        out[: a.shape[0]] = a
        return out

    w1T = pad128(c(np.asarray(inputs["w1"]).T))
    pmat = np.zeros((128, 168), mmf)
    pmat[0:64, 0:32] = np.asarray(inputs["w2"]).T       # [64, 32]
    pmat[0:32, 32:96] = np.asarray(inputs["gw1"]).T     # [32, 64]
    pmat[0:64, 96:160] = np.asarray(inputs["gw2"]).T    # [64, 64]
    pmat[0:64, 160:168] = np.asarray(inputs["gw3"]).T   # [64, 8]
    pvec = np.zeros((128, 9), f)
    pvec[:, 7] = EPS
    pvec[:, 8] = 1.0
    pvec[100:103, 8] = 100.0
    pvec[0:64, 0] = inputs["gamma1"]
    pvec[0:64, 1] = inputs["beta1"]
    pvec[0:32, 2] = inputs["gamma2"]
    pvec[0:32, 3] = inputs["beta2"]
    pvec[0:64, 4] = inputs["gb1"]
    pvec[0:64, 5] = inputs["gb2"]
    pvec[0:8, 6] = inputs["gb3"]
    ew1r = pad128(c(np.asarray(inputs["ew1"]).transpose(1, 0, 2).reshape(DIN, E * H)))
    ew1r[DIN, :] = np.asarray(inputs["eb1"], dtype=mmf).reshape(E * H)
    # [e, h, k] -> [p, j, c, i2, k] with e = 2j+i2, h = 128c+p: matmul (j, c)
    # reads the contiguous column block j*1024 + c*512.
    ew2r = c(
        np.asarray(inputs["ew2"])
        .reshape(4, 2, 2, 128, H)
        .transpose(3, 0, 2, 1, 4)
        .reshape(128, 2 * E * H)
    )
    ew3r = c(
        np.asarray(inputs["ew3"])
        .reshape(E, 2, 128, DOUT)
        .transpose(2, 1, 0, 3)
        .reshape(128, 2 * E * DOUT)
    )
    ebp = np.zeros((E, 308), mmf)
    ebp[:, 0:H] = np.asarray(inputs["eb2"], dtype=mmf)
    ebp[:, H : H + DOUT] = np.asarray(inputs["eb3"], dtype=mmf)

    shared = {
        "w1T": w1T, "pmat": pmat, "pvec": pvec,
        "ew1r": ew1r, "ew2r": ew2r, "ew3r": ew3r, "ebp": ebp,
    }
    in_maps = []
    for core in range(NCORES):
        xr = np.roll(x, -core * SH, axis=0)  # this core's shard -> rows 0:SH
        xt = pad128(c(xr.T))
        xt[DIN, :] = 1.0
        in_maps.append({**shared, "xT": xt})
    return in_maps


_prog = None


def _get_program():
    global _prog
    if _prog is None:
        _prog = build_program()
    return _prog


def kernel(**inputs) -> np.ndarray:
    nc = _get_program()
    in_maps = prep_in_maps(inputs)
    res = run_bass_kernel_spmd(nc, in_maps, core_ids=list(range(NCORES)))
    return np.concatenate(
        [np.asarray(res.results[cid]["out"]) for cid in range(NCORES)], axis=0
    )
